# revision 44
# baseline (speedup 1.0000x reference)
"""Trainium2 Bass kernel for nn_AttenLayer (ragged-sequence attention pooling).

Math (per batch b, with length L_b):
    proj   = tanh(nn_outs @ W^T + b)           # (S, A)
    scores = proj @ context                     # (S,)
    atten  = masked_softmax(scores, L_b)        # (S,), zeros beyond L_b
    out    = atten @ nn_outs                    # (H,)

Ragged slot-capped data-parallel sharding over 8 cores (see plan()):
batches sorted by length desc; slot k = ranks [8k, 8k+8), one batch per
core per slot; all cores run one SPMD instruction stream sized by the
per-slot width caps.

Phase-1 runs in fp8(e4m3) dual-row mode (2x PE throughput, 2x contraction
per instruction = 4x fewer cycles than bf16): x is quantized to e4m3*16
in an h-pair-interleaved layout, W^T to e4m3*512 in the SwInterleave
ldweights layout.  The fp8 quantization error is tamed by a linear
correction: scores += alpha * v^T x8 with v = (W - W8)^T c precomputed on
host and alpha ~ E[tanh'] = 0.5, which cancels the W-quantization error in
the locally-linear regime of tanh (measured rel err 0.0168 vs 0.0217
uncorrected; gate is 2e-2).  The correction is a zero-padded-stationary
fp8 DoubleRow matvec (1 cycle/token) accumulated directly into the scores
PSUM; to keep its output scale representable in e4m3, the whole scores
pipeline (ctx, mask) is scaled by 4096 and descaled in the softmax exp.

Scores matmul stays bf16 (proj fp8 would add ~1.8% error), phase 3
(atten @ nn_outs) stays bf16 (nat fp8 would add ~2.5%).

The p1 phase is ACT(tanh)-bound, so scheduling centers on keeping the
tanh stream dense: waves of 1-2 slots run smallest-first (see WAVE_SPANS)
sharing a [64, 512]-per-chunk scores PSUM via the zero-padded stationary
trick; scores matmuls trail their phase-1 producers by PIPE a-steps
(software pipeline over the tanh latency); each wave's mask-add + max
runs per-chunk inline behind the psum stops; the wave's PE finish work
(ex transposes + phase-3 matmuls, ex transposed on PE as phase-3 lhsT,
1/sum applied on the final [1, H] copy) is deferred onto a backlog of
small closures consumed inside later slots' p1 loops, filling PE slack
without stalling the in-order PE queue behind softmax chains.
"""

import sys

for _p in ("/opt/trn_rl_repo",):
    if _p not in sys.path:
        sys.path.insert(0, _p)

import numpy as np
import ml_dtypes

import concourse.bass as bass
from concourse import bacc
import concourse.mybir as mybir
import concourse.tile as tile
from concourse.masks import make_identity

B, S, H, A = 64, 2048, 512, 512
NCORES = 8
BPC = B // NCORES          # slots per core
# wave processing order: small slots first so their phase-3 work is
# available early and fills the PE gaps of the big ACT-bound p1 phases;
# the biggest pair runs third so its large phase-3 block overlaps the
# following medium wave, and the tiniest slot ends the kernel.
WAVE_SPANS = [(6, 1), (2, 2), (0, 2), (4, 2), (7, 1)]
WB = 4                     # ctx zero-pad layout width (max columns)
MW = 2                     # max wave size (rows in masks/softmax tiles)

AC = A // 128              # 4 a-chunks
HC = H // 128              # 4 h-chunks (2 dual-row pairs)

F32 = mybir.dt.float32
BF16 = mybir.dt.bfloat16
FP8 = mybir.dt.float8e4
E4M3 = ml_dtypes.float8_e4m3
DR = mybir.MatmulPerfMode.DoubleRow
DRSW = mybir.MatmulPerfMode.DoubleRowSwInterleave

SX = 16.0                  # x fp8 scale
SW = 512.0                 # W fp8 scale
SC = 4096.0                # scores-psum (ctx/mask) scale
ALPHA = 0.5                # linearization coefficient ~ E[tanh'(y)]
KV = SC * ALPHA / SX       # v fp8 scale
PIPE = 4                   # scores matmuls trail p1 by this many a-steps
VPAD = 64                  # dual-fp8 ldweights needs >=64 stationary cols


def build_nc(caps, widths, repeat: int = 1) -> bass.Bass:
    caps = [int(c) for c in caps]
    widths = [int(w) for w in widths]
    assert len(caps) == BPC and all(1 <= c <= S // 128 for c in caps)
    assert all(caps[i] >= caps[i + 1] for i in range(BPC - 1))
    assert all(widths[i] >= widths[i + 1] for i in range(BPC - 1))
    assert all((c - 1) * 128 < w <= c * 128 for c, w in zip(caps, widths))
    Wk = widths
    njs = [(w + 511) // 512 for w in Wk]     # 512-wide scores tiles
    xt_off = [0]
    nat_off = [0]
    for k in range(BPC):
        xt_off.append(xt_off[-1] + 4 * Wk[k])
        nat_off.append(nat_off[-1] + caps[k] * 512)

    nc = bacc.Bacc()

    xt_d = nc.declare_dram_parameter("xt", [128, xt_off[-1]], FP8, isOutput=False)
    nat_d = nc.declare_dram_parameter("nat", [128, nat_off[-1]], BF16, isOutput=False)
    # SwInterleave ldweights blocks per (a-chunk, h-pair): see make_in_maps
    wt_d = nc.declare_dram_parameter("wt", [128, AC * 2 * 256], FP8, isOutput=False)
    ctx_d = nc.declare_dram_parameter("ctx", [128, AC * WB * VPAD], BF16, isOutput=False)
    v_d = nc.declare_dram_parameter("v8", [128, 2 * WB * 2 * VPAD], FP8, isOutput=False)
    pb_d = nc.declare_dram_parameter("pb", [128, AC], F32, isOutput=False)
    # mask rows regrouped per wave so each wave's rows start at partition 0
    mask_d = nc.declare_dram_parameter(
        "mask", [MW, len(WAVE_SPANS) * S], BF16, isOutput=False
    )
    out_d = nc.declare_dram_parameter("out", [BPC, H], F32, isOutput=True)

    with tile.TileContext(nc) as tc:
        with (
            tc.tile_pool(name="const", bufs=1) as const_pool,
            tc.tile_pool(name="xt", bufs=3) as xt_pool,
            tc.tile_pool(name="nat", bufs=5) as nat_pool,
            tc.tile_pool(name="projT", bufs=10) as proj_pool,
            tc.tile_pool(name="smx", bufs=2) as smx_pool,
            tc.tile_pool(name="attT", bufs=8) as attT_pool,
            tc.tile_pool(name="osb", bufs=4) as os_pool,
            tc.tile_pool(name="p1ps", bufs=2, space="PSUM") as p1_psum,
            tc.tile_pool(name="scps", bufs=4, space="PSUM") as sc_psum,
            tc.tile_pool(name="atps", bufs=1, space="PSUM") as at_psum,
            tc.tile_pool(name="ops", bufs=1, space="PSUM") as out_psum,
        ):
            # ---- constants (DMAs deferred to first use in slot 0) ----
            wt_sb = const_pool.tile([128, AC * 2 * 256], FP8, tag="wt")
            ctx_sb = const_pool.tile([128, AC * WB * VPAD], BF16, tag="ctx")
            v_sb = const_pool.tile([128, 2, WB, 2, VPAD], FP8, tag="v8")
            pb_sb = const_pool.tile([128, AC], F32, tag="pb")
            ident = const_pool.tile([WB, WB], BF16, tag="ident")
            make_identity(nc, ident[:])
            nat_sb = {}

            mask_sb = const_pool.tile([MW, len(WAVE_SPANS) * S], BF16, tag="mask")
            scpss = {}  # w -> list of scores psum tiles [VPAD, 512]
            attT = {}   # (w, g) -> [128, 4*MW] bf16, col = cs*jj + bwi
            smxs = {}   # w -> (ex, rvT) from finish_softmax
            from collections import deque
            pe_backlog = deque()  # deferred transpose/p3 closures (PE work)
            # last slot-in-wave that writes scores tile j (512-granular)
            last_writer = {}
            for w, (b0, wb) in enumerate(WAVE_SPANS):
                last_writer[w] = [
                    max(bwi for bwi in range(wb) if njs[b0 + bwi] > j)
                    for j in range(njs[b0])
                ]
            consts_loaded = [False]

            def trace_slot(w, b0, wb, bwi):
                k = b0 + bwi
                W = Wk[k]
                xt = xt_pool.tile([128, 2, 2, 2048], FP8, tag="xt")
                src = xt_d[:, xt_off[k] : xt_off[k] + 4 * W].rearrange(
                    "p (q i w) -> p q i w", q=2, i=2
                )
                if not consts_loaded[0]:
                    consts_loaded[0] = True
                    # a=0's weight blocks first so the first p1 matmul can
                    # start as soon as the first xt lands
                    nc.sync.dma_start(wt_sb[:, :512], wt_d[:, :512])
                    nc.gpsimd.dma_start(pb_sb[:], pb_d[:])
                    nc.gpsimd.dma_start(
                        v_sb[:].rearrange("p a b c d -> p (a b c d)"), v_d[:]
                    )
                    nc.gpsimd.dma_start(ctx_sb[:], ctx_d[:])
                if bwi == 0:
                    # this wave's mask rows, just in time for its softmax
                    nc.gpsimd.dma_start(
                        mask_sb[:, w * S : (w + 1) * S],
                        mask_d[:, w * S : (w + 1) * S],
                    )
                if w <= 1 and W > 1024:
                    # early big slots: land the first p1 chunks fast
                    for c0 in range(0, W, 1024):
                        c1 = min(W, c0 + 1024)
                        nc.sync.dma_start(
                            xt[:, :, :, c0:c1], src[:, :, :, c0:c1]
                        )
                else:
                    nc.sync.dma_start(xt[:, :, :, :W], src[:])
                if bwi == 0 and w == 0:
                    nc.sync.dma_start(wt_sb[:, 512:], wt_d[:, 512:])
                natk = nat_pool.tile([128, 16 * 512], BF16, tag="nat")
                nat_sb[k] = natk
                # nat is phase-3 data (needed late): keep it all on the Pool
                # queue so xt flows without queueing delay on SP
                nc.gpsimd.dma_start(
                    natk[:, : caps[k] * 512],
                    nat_d[:, nat_off[k] : nat_off[k + 1]],
                )
                # the last slot of a wave interleaves the wave's per-chunk
                # mask-add + max (DVE) right behind each chunk's psum stop,
                # so only the exp chain remains after the wave's last tanh
                chunk_fin = None
                if bwi == wb - 1:
                    scm = smx_pool.tile([MW, S], F32, tag="scm")
                    pmax = smx_pool.tile([MW, 4], F32, tag="pmax")
                    smxs[w] = {"scm": scm, "pmax": pmax}
                    Wmax = Wk[b0]
                    mwave = mask_sb[:wb, w * S : (w + 1) * S]

                    def chunk_fin(j, wb=wb, Wmax=Wmax, mwave=mwave, w=w,
                                  scm=scm, pmax=pmax):
                        wjj = min(512, Wmax - j * 512)
                        sl = slice(j * 512, j * 512 + wjj)
                        nc.vector.tensor_tensor(
                            out=scm[:wb, sl], in0=scpss[w][j][:wb, :wjj],
                            in1=mwave[:, sl], op=mybir.AluOpType.add,
                        )
                        nc.vector.reduce_max(
                            pmax[:wb, j : j + 1], scm[:wb, sl],
                            axis=mybir.AxisListType.X,
                        )

                pq = []  # software-pipeline: scores MMs trail p1 by PIPE steps
                for jt in range(njs[k]):
                    base = jt * 512
                    wj5 = min(512, W - base)
                    halves = [(0, min(256, wj5))]
                    if wj5 > 256:
                        halves.append((256, wj5 - 256))
                    # fp8 correction matvec: scores[bwi] += SC*alpha*v^T x
                    # (zero-padded stationary; carries the tile's psum start)
                    for ho, hw in halves:
                        for q in range(2):
                            nc.tensor.matmul(
                                scpss[w][jt][:, ho : ho + hw],
                                v_sb[:, q, bwi, :, :],
                                xt[:, q, :, base + ho : base + ho + hw],
                                start=(bwi == 0 and ho == 0 and q == 0),
                                stop=False,
                                perf_mode=DR,
                            )
                    for a in range(AC):
                        ps = p1_psum.tile([128, 512], F32, tag="p1")
                        for hi, (ho, hw) in enumerate(halves):
                            for q in range(2):
                                nc.tensor.matmul(
                                    ps[:, ho : ho + hw],
                                    wt_sb[:, (a * 2 + q) * 256 : (a * 2 + q + 1) * 256],
                                    xt[:, q, :, base + ho : base + ho + hw],
                                    start=(ho == 0 and q == 0),
                                    stop=(hi == len(halves) - 1 and q == 1),
                                    perf_mode=DRSW,
                                )
                        if len(pq) >= PIPE:
                            pq.pop(0)()
                        pt = proj_pool.tile([128, 512], BF16, tag="projT")
                        nc.scalar.activation(
                            pt[:, :wj5],
                            ps[:, :wj5],
                            mybir.ActivationFunctionType.Tanh,
                            bias=pb_sb[:, a : a + 1],
                            scale=1.0 / (SX * SW),
                        )
                        # ctx col bwi is context's a-chunk (x SC), others zero:
                        # only row bwi of the wave's scores psum accumulates.
                        # stationary padded to VPAD cols so start/stop cover
                        # the same 64 psum partitions as the v matmuls.
                        last = bwi == last_writer[w][jt] and a == AC - 1
                        def pend(pt=pt, jt=jt, a=a, wj5=wj5, last=last):
                            nc.tensor.matmul(
                                scpss[w][jt][:, :wj5],
                                ctx_sb[:, (a * WB + bwi) * VPAD : (a * WB + bwi + 1) * VPAD],
                                pt[:, :wj5],
                                start=False,
                                stop=last,
                            )
                        pq.append(pend)
                        # fill the p1 stream's PE slack with deferred
                        # transpose/p3 closures from earlier waves: one per
                        # a-pair so the ACT pipeline never starves, every
                        # a-step once the backlog is long (late waves)
                        if pe_backlog and (a % 2 == 1 or len(pe_backlog) > 12):
                            pe_backlog.popleft()()
                    # chunk jt-1's scores psum group closed during this jt's
                    # trailing ctx matmuls
                    if chunk_fin is not None and jt > 0:
                        chunk_fin(jt - 1)
                for f in pq:
                    f()
                if chunk_fin is not None:
                    # this slot's final chunk, plus any wider chunks whose
                    # last writer was an earlier (wider) slot
                    for j in range(njs[k] - 1, njs[b0]):
                        chunk_fin(j)

            def finish_softmax(w, b0, wb):
                Wmax = Wk[b0]
                njw = njs[b0]
                capm = caps[b0]
                # per-chunk masked scores + maxes were computed inline by the
                # wave's last trace_slot; only the exp chain remains here
                scm = smxs[w]["scm"]
                pmax = smxs[w]["pmax"]
                mx = smx_pool.tile([wb, 1], F32, tag="mx")
                nc.vector.reduce_max(
                    mx[:], pmax[:wb, :njw], axis=mybir.AxisListType.X, negate=True
                )
                mx2 = smx_pool.tile([wb, 1], F32, tag="mx2")
                nc.vector.tensor_scalar_mul(mx2[:], mx[:], 1.0 / SC)
                ex = smx_pool.tile([MW, S], BF16, tag="ex")
                rs = smx_pool.tile([wb, 1], F32, tag="rs")
                nc.scalar.activation(
                    ex[:wb, :Wmax],
                    scm[:wb, :Wmax],
                    mybir.ActivationFunctionType.Exp,
                    bias=mx2[:],
                    scale=1.0 / SC,
                    accum_out=rs[:],
                )
                rv = smx_pool.tile([wb, 1], F32, tag="rv")
                nc.vector.reciprocal(rv[:], rs[:])
                if wb > 1:
                    # move rv to partition 0 (tiny SBUF->SBUF DMA) so the
                    # final output copy can read it as a per-partition scale
                    rvT = smx_pool.tile([1, MW], F32, tag="rvT")
                    nc.gpsimd.dma_start(rvT[0:1, :wb], rv[:, 0:1])
                else:
                    rvT = rv  # single-row wave: already at partition 0
                # zero the ex tail beyond the wave width so the last
                # 128-chunk transposes clean zeros
                if capm * 128 > Wmax:
                    nc.vector.memset(ex[:wb, Wmax : capm * 128], 0.0)
                smxs[w]["ex"] = ex
                smxs[w]["rvT"] = rvT

            def push_finish_pe(w, b0, wb):
                """Queue the wave's PE work (ex transposes + phase 3) as
                small closures consumed inside later slots' p1 loops."""
                capm = caps[b0]
                ex, rvT = smxs[w]["ex"], smxs[w]["rvT"]
                # column stride even so bf16 PSUM offsets stay 4B-aligned
                cs = 2
                for g in range((capm + 3) // 4):
                    def tclos(g=g, wb=wb, ex=ex, w=w):
                        na = min(4, capm - 4 * g)
                        aps = at_psum.tile([128, 4 * MW], BF16, tag="atps")
                        for jj in range(na):
                            ch = 4 * g + jj
                            nc.tensor.transpose(
                                aps[:, jj * cs : jj * cs + wb],
                                ex[:wb, ch * 128 : (ch + 1) * 128],
                                ident[:wb, :wb],
                            )
                        att_sb = attT_pool.tile([128, 4 * MW], BF16, tag="attT")
                        nc.vector.tensor_copy(
                            att_sb[:, : na * cs].rearrange(
                                "p (n c) -> p n c", c=cs
                            )[:, :, :wb],
                            aps[:, : na * cs].rearrange(
                                "p (n c) -> p n c", c=cs
                            )[:, :, :wb],
                        )
                        attT[(w, g)] = att_sb
                    pe_backlog.append(tclos)
                # phase 3: out[k] = (sum_s ex[s] * x[s, :]) * rv
                for bwi in reversed(range(wb)):
                    k = b0 + bwi
                    holder = {}
                    for n0 in range(0, caps[k], 2):
                        def p3clos(n0=n0, k=k, bwi=bwi, w=w, holder=holder):
                            if n0 == 0:
                                holder["ops"] = out_psum.tile(
                                    [1, H], F32, tag="ops", name="ops"
                                )
                            ops = holder["ops"]
                            for n in range(n0, min(n0 + 2, caps[k])):
                                col = (n % 4) * cs + bwi
                                nc.tensor.matmul(
                                    ops[:],
                                    attT[(w, n // 4)][:, col : col + 1],
                                    nat_sb[k][:, n * 512 : (n + 1) * 512],
                                    start=(n == 0),
                                    stop=(n == caps[k] - 1),
                                )
                        pe_backlog.append(p3clos)
                    def fin(k=k, bwi=bwi, rvT=rvT, holder=holder):
                        os_b = os_pool.tile([1, H], F32, tag="os")
                        nc.vector.tensor_scalar_mul(
                            os_b[:], holder["ops"][:], rvT[0:1, bwi : bwi + 1]
                        )
                        nc.gpsimd.dma_start(out_d[k : k + 1, :], os_b[:])
                    pe_backlog.append(fin)

            for _rep in range(repeat):
                scpss.clear()
                attT.clear()
                smxs.clear()
                for w, (b0, wb) in enumerate(WAVE_SPANS):
                    scpss[w] = [
                        sc_psum.tile([VPAD, 512], F32, tag="scps", name="scps")
                        for _j in range(njs[b0])
                    ]
                    for bwi in range(wb):
                        trace_slot(w, b0, wb, bwi)
                    # softmax (DVE/ACT) fires immediately after the wave's
                    # last tanh; the PE part queues onto the backlog
                    finish_softmax(w, b0, wb)
                    push_finish_pe(w, b0, wb)
                while pe_backlog:
                    pe_backlog.popleft()()

    nc.finalize()
    return nc


_NC_CACHE = {}


def get_nc(caps, widths, repeat: int = 1) -> bass.Bass:
    key = (tuple(caps), tuple(widths), repeat)
    if key not in _NC_CACHE:
        _NC_CACHE[key] = build_nc(caps, widths, repeat=repeat)
    return _NC_CACHE[key]


def plan(lens):
    """Sort batches by length desc; slot k = ranks [8k, 8k+8), one per core.

    Window maxima of the descending sort minimize the summed per-slot caps
    (both the per-token p1/scores work and the 512*ceil(L/128) p3 term are
    monotone in L, so length-sorting is optimal for the combined cost).
    """
    lens = np.asarray(lens).reshape(B).astype(np.int64)
    order = np.argsort(-lens, kind="stable")
    assign = order.reshape(BPC, NCORES)          # [slot, core] -> batch
    widths = [int(lens[assign[k, 0]]) for k in range(BPC)]
    caps = [(w + 127) // 128 for w in widths]
    return caps, widths, assign


def make_in_maps(nn_outs, batch_lens, context, proj_w, proj_b, caps, widths,
                 assign):
    xf = np.asarray(nn_outs, dtype=np.float32)
    x_bf = xf.astype(ml_dtypes.bfloat16)
    x8 = (xf * SX).astype(E4M3)                  # [B, S, H] fp8
    lens = np.asarray(batch_lens).reshape(B).astype(np.int64)
    ctx_f = np.asarray(context, np.float32)
    Wf = np.asarray(proj_w, np.float32)          # [A, H]

    # --- W8 in SwInterleave ldweights layout, per (a-chunk, h-pair) ---
    W8 = (Wf * SW).astype(E4M3)
    # logical L[a,q][k,i,m] = W8[128a+m, 128(2q+i)+k]; storage st[k, 2j+i] =
    # L[k, i, 127-j] (pair-interleaved, columns reversed)
    Lf = np.ascontiguousarray(W8).reshape(AC, 128, 2, 2, 128)  # [a,m,q,i,k]
    T = Lf.transpose(0, 2, 4, 3, 1)                            # [a,q,k,i,m]
    st = T[..., ::-1].transpose(0, 1, 2, 4, 3)                 # [a,q,k,j,i]
    wt_host = np.ascontiguousarray(
        st.reshape(AC * 2, 128, 256).transpose(1, 0, 2).reshape(128, AC * 2 * 256)
    )

    # --- correction vector v = (W - W8)^T c, fp8, zero-padded stationary ---
    v = (Wf - W8.astype(np.float32) / SW).T @ ctx_f            # [H]
    v8 = (v * KV).astype(E4M3)
    vh = np.zeros((128, 2, WB, 2, VPAD), E4M3)
    vq = v8.reshape(2, 2, 128)                                 # [q, i, p]
    for bwi in range(WB):
        vh[:, :, bwi, :, bwi] = vq.transpose(2, 0, 1)
    v_host = np.ascontiguousarray(vh.reshape(128, 2 * WB * 2 * VPAD))

    # --- ctx (x SC) in the zero-padded wave layout ---
    ctx_c = (ctx_f * SC).reshape(AC, 128)
    ctx_host = np.zeros((128, AC, WB, VPAD), np.float32)
    for a in range(AC):
        for bw in range(WB):
            ctx_host[:, a, bw, bw] = ctx_c[a]
    ctx_host = np.ascontiguousarray(
        ctx_host.reshape(128, AC * WB * VPAD)
    ).astype(ml_dtypes.bfloat16)
    pb_host = np.ascontiguousarray(
        np.asarray(proj_b, np.float32).reshape(AC, 128).T
    )
    iota = np.arange(S)[None, :]

    xt_w = sum(4 * w for w in widths)
    nat_w = sum(c * 512 for c in caps)
    in_maps = []
    for c in range(NCORES):
        xt_all = np.empty((128, xt_w), E4M3)
        nat_all = np.empty((128, nat_w), ml_dtypes.bfloat16)
        mask = np.zeros((MW, len(WAVE_SPANS) * S), ml_dtypes.bfloat16)
        xo = no = 0
        for k in range(BPC):
            b = assign[k, c]
            W = widths[k]
            # xt8[p, q, i, s] = x8[b, s, 128(2q+i)+p]
            xt_all[:, xo : xo + 4 * W] = (
                x8[b, :W, :].T.reshape(2, 2, 128, W)
                .transpose(2, 0, 1, 3).reshape(128, 4 * W)
            )
            xo += 4 * W
            Wc = caps[k] * 128
            nat_all[:, no : no + caps[k] * 512] = (
                x_bf[b, :Wc, :].reshape(caps[k], 128, 512).transpose(1, 0, 2)
                .reshape(128, caps[k] * 512)
            )
            no += caps[k] * 512
        for w, (b0, wb) in enumerate(WAVE_SPANS):
            for bwi in range(wb):
                b = assign[b0 + bwi, c]
                mask[bwi, w * S : (w + 1) * S] = np.where(
                    iota[0] < lens[b], 0.0, -30000.0 * SC
                )
        in_maps.append(
            {
                "xt": xt_all,
                "nat": nat_all,
                "wt": wt_host,
                "ctx": ctx_host,
                "v8": v_host,
                "pb": pb_host,
                "mask": mask,
            }
        )
    return in_maps


def run(nn_outs, batch_lens, context, proj_w, proj_b, trace=False, repeat=1,
        **trace_kw):
    from concourse.bass_utils import run_bass_kernel_spmd

    caps, widths, assign = plan(batch_lens)
    nc = get_nc(caps, widths, repeat=repeat)
    in_maps = make_in_maps(
        nn_outs, batch_lens, context, proj_w, proj_b, caps, widths, assign
    )
    res = run_bass_kernel_spmd(
        nc, in_maps, list(range(NCORES)), trace=trace, **trace_kw
    )
    out = np.empty((B, H), np.float32)
    for c in range(NCORES):
        out[assign[:, c]] = res.results[c]["out"]
    return out, res


def kernel(nn_outs, batch_lens, context, proj_w, proj_b):
    out, _ = run(nn_outs, batch_lens, context, proj_w, proj_b, trace=False)
    return out


# revision 49
# speedup vs baseline: 1.0502x; 1.0502x over previous
"""Trainium2 Bass kernel for nn_AttenLayer (ragged-sequence attention pooling).

Math (per batch b, with length L_b):
    proj   = tanh(nn_outs @ W^T + b)           # (S, A)
    scores = proj @ context                     # (S,)
    atten  = masked_softmax(scores, L_b)        # (S,), zeros beyond L_b
    out    = atten @ nn_outs                    # (H,)

Ragged slot-capped data-parallel sharding over 8 cores (see plan()):
batches sorted by length desc; slot k = ranks [8k, 8k+8), one batch per
core per slot; all cores run one SPMD instruction stream sized by the
per-slot width caps.

Phase-1 runs in fp8(e4m3) dual-row mode (2x PE throughput, 2x contraction
per instruction = 4x fewer cycles than bf16): x is quantized to e4m3*16
in an h-pair-interleaved layout, W^T to e4m3*512 in the SwInterleave
ldweights layout.  The fp8 quantization error is tamed by a linear
correction: scores += alpha * v^T x8 with v = (W - W8)^T c precomputed on
host and alpha ~ E[tanh'] = 0.5, which cancels the W-quantization error in
the locally-linear regime of tanh (measured rel err 0.0168 vs 0.0217
uncorrected; gate is 2e-2).  The correction is a zero-padded-stationary
fp8 DoubleRow matvec (1 cycle/token) accumulated directly into the scores
PSUM; to keep its output scale representable in e4m3, the whole scores
pipeline (ctx, mask) is scaled by 4096 and descaled in the softmax exp.

Scores matmul stays bf16 (proj fp8 would add ~1.8% error), phase 3
(atten @ nn_outs) stays bf16 (nat fp8 would add ~2.5%).

The p1 phase is ACT(tanh)-bound, so scheduling centers on keeping the
tanh stream dense: waves of 1-2 slots run smallest-first (see WAVE_SPANS)
sharing a [64, 512]-per-chunk scores PSUM via the zero-padded stationary
trick; scores matmuls trail their phase-1 producers by PIPE a-steps
(software pipeline over the tanh latency); each wave's mask-add + max
runs per-chunk inline behind the psum stops; the wave's PE finish work
(ex transposes + phase-3 matmuls, ex transposed on PE as phase-3 lhsT,
1/sum applied on the final [1, H] copy) is deferred onto a backlog of
small closures consumed inside later slots' p1 loops, filling PE slack
without stalling the in-order PE queue behind softmax chains.
"""

import sys

for _p in ("/opt/trn_rl_repo",):
    if _p not in sys.path:
        sys.path.insert(0, _p)

import numpy as np
import ml_dtypes

import concourse.bass as bass
from concourse import bacc
import concourse.mybir as mybir
import concourse.tile as tile
from concourse.masks import make_identity

B, S, H, A = 64, 2048, 512, 512
NCORES = 8
BPC = B // NCORES          # slots per core
# wave processing order: small slots first so their phase-3 work is
# available early and fills the PE gaps of the big ACT-bound p1 phases;
# the biggest pair runs third so its large phase-3 block overlaps the
# following medium wave, and the tiniest slot ends the kernel.
WAVE_SPANS = [(6, 1), (2, 1), (0, 1), (1, 1), (3, 1), (4, 1), (5, 1), (7, 1)]
WB = 4                     # ctx zero-pad layout width (max columns)
MW = 2                     # max wave size (rows in masks/softmax tiles)

AC = A // 128              # 4 a-chunks
HC = H // 128              # 4 h-chunks (2 dual-row pairs)

F32 = mybir.dt.float32
BF16 = mybir.dt.bfloat16
FP8 = mybir.dt.float8e4
E4M3 = ml_dtypes.float8_e4m3
DR = mybir.MatmulPerfMode.DoubleRow
DRSW = mybir.MatmulPerfMode.DoubleRowSwInterleave

SX = 16.0                  # x fp8 scale
SW = 512.0                 # W fp8 scale
SC = 4096.0                # scores-psum (ctx/mask) scale
ALPHA = 0.5                # linearization coefficient ~ E[tanh'(y)]
KV = SC * ALPHA / SX       # v fp8 scale
PIPE = 2                   # scores closures trail p1 by this many (jt,a)-steps
VPAD = 64                  # dual-fp8 ldweights needs >=64 stationary cols


def build_nc(caps, widths, repeat: int = 1) -> bass.Bass:
    caps = [int(c) for c in caps]
    widths = [int(w) for w in widths]
    assert len(caps) == BPC and all(1 <= c <= S // 128 for c in caps)
    assert all(caps[i] >= caps[i + 1] for i in range(BPC - 1))
    assert all(widths[i] >= widths[i + 1] for i in range(BPC - 1))
    assert all((c - 1) * 128 < w <= c * 128 for c, w in zip(caps, widths))
    Wk = widths
    njs = [(w + 511) // 512 for w in Wk]     # 512-wide scores tiles
    xt_off = [0]
    nat_off = [0]
    for k in range(BPC):
        xt_off.append(xt_off[-1] + 4 * Wk[k])
        nat_off.append(nat_off[-1] + caps[k] * 512)

    nc = bacc.Bacc()

    xt_d = nc.declare_dram_parameter("xt", [128, xt_off[-1]], FP8, isOutput=False)
    nat_d = nc.declare_dram_parameter("nat", [128, nat_off[-1]], BF16, isOutput=False)
    # SwInterleave ldweights blocks per (a-chunk, h-pair): see make_in_maps
    wt_d = nc.declare_dram_parameter("wt", [128, AC * 2 * 256], FP8, isOutput=False)
    ctx_d = nc.declare_dram_parameter("ctx", [128, AC * WB * VPAD], BF16, isOutput=False)
    v_d = nc.declare_dram_parameter("v8", [128, 2 * WB * 2 * VPAD], FP8, isOutput=False)
    pb_d = nc.declare_dram_parameter("pb", [128, AC], F32, isOutput=False)
    # mask rows regrouped per wave so each wave's rows start at partition 0
    mask_d = nc.declare_dram_parameter(
        "mask", [MW, len(WAVE_SPANS) * S], BF16, isOutput=False
    )
    out_d = nc.declare_dram_parameter("out", [BPC, H], F32, isOutput=True)

    with tile.TileContext(nc) as tc:
        with (
            tc.tile_pool(name="const", bufs=1) as const_pool,
            tc.tile_pool(name="xt", bufs=3) as xt_pool,
            tc.tile_pool(name="nat", bufs=5) as nat_pool,
            tc.tile_pool(name="projT", bufs=6) as proj_pool,
            tc.tile_pool(name="smx", bufs=2) as smx_pool,
            tc.tile_pool(name="attT", bufs=8) as attT_pool,
            tc.tile_pool(name="osb", bufs=4) as os_pool,
            tc.tile_pool(name="p1ps", bufs=2, space="PSUM") as p1_psum,
            tc.tile_pool(name="scps", bufs=2, space="PSUM") as sc_psum,
            tc.tile_pool(name="atps", bufs=1, space="PSUM") as at_psum,
            tc.tile_pool(name="ops", bufs=1, space="PSUM") as out_psum,
        ):
            # ---- constants (DMAs deferred to first use in slot 0) ----
            wt_sb = const_pool.tile([128, AC * 2 * 256], FP8, tag="wt")
            ctx_sb = const_pool.tile([128, AC * WB * VPAD], BF16, tag="ctx")
            v_sb = const_pool.tile([128, 2, WB, 2, VPAD], FP8, tag="v8")
            pb_sb = const_pool.tile([128, AC], F32, tag="pb")
            ident = const_pool.tile([WB, WB], BF16, tag="ident")
            make_identity(nc, ident[:])
            nat_sb = {}

            mask_sb = const_pool.tile([MW, len(WAVE_SPANS) * S], BF16, tag="mask")
            attT = {}   # (w, g) -> [128, 4*MW] bf16, col = cs*jj + bwi
            smxs = {}   # w -> per-wave softmax tiles
            from collections import deque
            pe_backlog = deque()  # deferred transpose/p3 closures (PE work)
            consts_loaded = [False]

            def trace_slot(w, b0, wb, bwi):
                k = b0 + bwi
                W = Wk[k]
                xt = xt_pool.tile([128, 2, 2, 2048], FP8, tag="xt")
                src = xt_d[:, xt_off[k] : xt_off[k] + 4 * W].rearrange(
                    "p (q i w) -> p q i w", q=2, i=2
                )
                if not consts_loaded[0]:
                    consts_loaded[0] = True
                    # a=0's weight blocks first so the first p1 matmul can
                    # start as soon as the first xt lands
                    nc.sync.dma_start(wt_sb[:, :512], wt_d[:, :512])
                    nc.gpsimd.dma_start(pb_sb[:], pb_d[:])
                    nc.gpsimd.dma_start(
                        v_sb[:].rearrange("p a b c d -> p (a b c d)"), v_d[:]
                    )
                    nc.gpsimd.dma_start(ctx_sb[:], ctx_d[:])
                if bwi == 0:
                    # this wave's mask rows, just in time for its softmax
                    nc.gpsimd.dma_start(
                        mask_sb[:, w * S : (w + 1) * S],
                        mask_d[:, w * S : (w + 1) * S],
                    )
                if w <= 1 and W > 1024:
                    # early big slots: land the first p1 chunks fast
                    for c0 in range(0, W, 1024):
                        c1 = min(W, c0 + 1024)
                        nc.sync.dma_start(
                            xt[:, :, :, c0:c1], src[:, :, :, c0:c1]
                        )
                else:
                    nc.sync.dma_start(xt[:, :, :, :W], src[:])
                if bwi == 0 and w == 0:
                    nc.sync.dma_start(wt_sb[:, 512:], wt_d[:, 512:])
                natk = nat_pool.tile([128, 16 * 512], BF16, tag="nat")
                nat_sb[k] = natk
                # nat is phase-3 data (needed late): keep it all on the Pool
                # queue so xt flows without queueing delay on SP
                nc.gpsimd.dma_start(
                    natk[:, : caps[k] * 512],
                    nat_d[:, nat_off[k] : nat_off[k + 1]],
                )
                # per-slot 512-wide score psum tiles (2-bank ring): each slot
                # drains its own row into the wave's shared scm right after
                # its ctx stops, so only the exp chain remains per wave.
                # the wave's scm/pmax SBUF tiles are shared across its slots.
                if bwi == 0:
                    smxs[w] = {
                        "scm": smx_pool.tile([MW, S], F32, tag="scm", name="scm"),
                        "pmax": smx_pool.tile([MW, 4], F32, tag="pmax", name="pmax"),
                    }
                    nc.vector.memset(smxs[w]["pmax"][:], -3.0e38)
                    # narrow rows' scm tails up to the wave width would be
                    # uninitialized: kill them now (row 0's drains overwrite
                    # its real region later)
                    if wb > 1 and Wk[b0] > Wk[b0 + wb - 1]:
                        nc.vector.memset(
                            smxs[w]["scm"][:wb, Wk[b0 + wb - 1] : Wk[b0]],
                            -3.0e9 * SC,
                        )
                scm = smxs[w]["scm"]
                pmax = smxs[w]["pmax"]
                scd = {}      # j512 -> this slot's score psum tile

                def chunk_fin(j):
                    wjj = min(512, W - j * 512)
                    sl = slice(j * 512, j * 512 + wjj)
                    nc.vector.tensor_tensor(
                        out=scm[bwi : bwi + 1, sl],
                        in0=scd[j][bwi : bwi + 1, :wjj],
                        in1=mask_sb[bwi : bwi + 1,
                                    w * S + j * 512 : w * S + j * 512 + wjj],
                        op=mybir.AluOpType.add,
                    )
                    nc.vector.reduce_max(
                        pmax[bwi : bwi + 1, j : j + 1],
                        scm[bwi : bwi + 1, sl],
                        axis=mybir.AxisListType.X,
                    )

                NJ4 = (W + 1023) // 1024
                pq = []  # scores closures trail p1 by PIPE (jt,a)-steps
                for jt in range(NJ4):
                    b4 = jt * 1024
                    wj4 = min(1024, W - b4)
                    # 512-wide regions of the 2-bank p1 psum tile
                    regions = [(0, min(512, wj4))]
                    if wj4 > 512:
                        regions.append((512, wj4 - 512))
                    for a in range(AC):
                        if len(pq) >= PIPE:
                            pq.pop(0)()
                        if a == 1 and jt > 0:
                            # previous iter's chunks: their stops popped at
                            # this step; drain now, one step before their
                            # psum ring slots get reused
                            for j in range(2 * (jt - 1), 2 * jt):
                                if j in scd:
                                    chunk_fin(j)
                        ps = p1_psum.tile([128, 1024], F32, tag="p1")
                        for ro, rw in regions:
                            nq = (rw + 255) // 256
                            for qi in range(nq):
                                qo = ro + qi * 256
                                qw = min(256, rw - qi * 256)
                                for q in range(2):
                                    nc.tensor.matmul(
                                        ps[:, qo : qo + qw],
                                        wt_sb[:, (a * 2 + q) * 256 : (a * 2 + q + 1) * 256],
                                        xt[:, q, :, b4 + qo : b4 + qo + qw],
                                        start=(qi == 0 and q == 0),
                                        stop=(qi == nq - 1 and q == 1),
                                        perf_mode=DRSW,
                                    )
                        pt = proj_pool.tile([128, 1024], BF16, tag="projT")
                        nc.scalar.activation(
                            pt[:, :wj4],
                            ps[:, :wj4],
                            mybir.ActivationFunctionType.Tanh,
                            bias=pb_sb[:, a : a + 1],
                            scale=1.0 / (SX * SW),
                        )
                        # ctx col bwi is context's a-chunk (x SC), others
                        # zero, padded to VPAD cols to match the v matmuls'
                        # psum partitions.  a==0 allocates+starts the score
                        # tiles; a==1 adds the fp8 v-correction matvecs.
                        def pend(pt=pt, b4=b4, a=a, regions=regions, xt=xt):
                            for ro, rw in regions:
                                j = (b4 + ro) // 512
                                if a == 0:
                                    scd[j] = sc_psum.tile(
                                        [VPAD, 512], F32, tag="scps",
                                        name="scps",
                                    )
                                nc.tensor.matmul(
                                    scd[j][:, :rw],
                                    ctx_sb[:, (a * WB + bwi) * VPAD : (a * WB + bwi + 1) * VPAD],
                                    pt[:, ro : ro + rw],
                                    start=(a == 0),
                                    stop=(a == AC - 1),
                                )
                                if a == 1:
                                    for qo in range(0, rw, 256):
                                        qw = min(256, rw - qo)
                                        for q in range(2):
                                            nc.tensor.matmul(
                                                scd[j][:, qo : qo + qw],
                                                v_sb[:, q, bwi, :, :],
                                                xt[:, q, :, b4 + ro + qo : b4 + ro + qo + qw],
                                                start=False,
                                                stop=False,
                                                perf_mode=DR,
                                            )
                        pq.append(pend)
                        # fill PE slack with deferred transpose/p3 closures
                        if pe_backlog and (a % 2 == 1 or len(pe_backlog) > 12):
                            pe_backlog.popleft()()
                for f in pq:
                    f()
                for j in range(max(0, 2 * (NJ4 - 1)), njs[k]):
                    if j in scd:
                        chunk_fin(j)


            def finish_softmax(w, b0, wb):
                Wmax = Wk[b0]
                njw = njs[b0]
                capm = caps[b0]
                # per-chunk masked scores + maxes were computed inline by the
                # wave's last trace_slot; only the exp chain remains here
                scm = smxs[w]["scm"]
                pmax = smxs[w]["pmax"]
                mx = smx_pool.tile([wb, 1], F32, tag="mx")
                nc.vector.reduce_max(
                    mx[:], pmax[:wb, :njw], axis=mybir.AxisListType.X, negate=True
                )
                mx2 = smx_pool.tile([wb, 1], F32, tag="mx2")
                nc.vector.tensor_scalar_mul(mx2[:], mx[:], 1.0 / SC)
                ex = smx_pool.tile([MW, S], BF16, tag="ex")
                rs = smx_pool.tile([wb, 1], F32, tag="rs")
                nc.scalar.activation(
                    ex[:wb, :Wmax],
                    scm[:wb, :Wmax],
                    mybir.ActivationFunctionType.Exp,
                    bias=mx2[:],
                    scale=1.0 / SC,
                    accum_out=rs[:],
                )
                rv = smx_pool.tile([wb, 1], F32, tag="rv")
                nc.vector.reciprocal(rv[:], rs[:])
                if wb > 1:
                    # move rv to partition 0 (tiny SBUF->SBUF DMA) so the
                    # final output copy can read it as a per-partition scale
                    rvT = smx_pool.tile([1, MW], F32, tag="rvT")
                    nc.gpsimd.dma_start(rvT[0:1, :wb], rv[:, 0:1])
                else:
                    rvT = rv  # single-row wave: already at partition 0
                # zero the ex tail beyond the wave width so the last
                # 128-chunk transposes clean zeros
                if capm * 128 > Wmax:
                    nc.vector.memset(ex[:wb, Wmax : capm * 128], 0.0)
                smxs[w]["ex"] = ex
                smxs[w]["rvT"] = rvT

            def push_finish_pe(w, b0, wb):
                """Queue the wave's PE work (ex transposes + phase 3) as
                small closures consumed inside later slots' p1 loops."""
                capm = caps[b0]
                ex, rvT = smxs[w]["ex"], smxs[w]["rvT"]
                # column stride even so bf16 PSUM offsets stay 4B-aligned
                cs = 2
                for g in range((capm + 3) // 4):
                    def tclos(g=g, wb=wb, ex=ex, w=w):
                        na = min(4, capm - 4 * g)
                        aps = at_psum.tile([128, 4 * MW], BF16, tag="atps")
                        for jj in range(na):
                            ch = 4 * g + jj
                            nc.tensor.transpose(
                                aps[:, jj * cs : jj * cs + wb],
                                ex[:wb, ch * 128 : (ch + 1) * 128],
                                ident[:wb, :wb],
                            )
                        att_sb = attT_pool.tile([128, 4 * MW], BF16, tag="attT")
                        nc.vector.tensor_copy(
                            att_sb[:, : na * cs].rearrange(
                                "p (n c) -> p n c", c=cs
                            )[:, :, :wb],
                            aps[:, : na * cs].rearrange(
                                "p (n c) -> p n c", c=cs
                            )[:, :, :wb],
                        )
                        attT[(w, g)] = att_sb
                    pe_backlog.append(tclos)
                # phase 3: out[k] = (sum_s ex[s] * x[s, :]) * rv
                for bwi in reversed(range(wb)):
                    k = b0 + bwi
                    holder = {}
                    for n0 in range(0, caps[k], 2):
                        def p3clos(n0=n0, k=k, bwi=bwi, w=w, holder=holder):
                            if n0 == 0:
                                holder["ops"] = out_psum.tile(
                                    [1, H], F32, tag="ops", name="ops"
                                )
                            ops = holder["ops"]
                            for n in range(n0, min(n0 + 2, caps[k])):
                                col = (n % 4) * cs + bwi
                                nc.tensor.matmul(
                                    ops[:],
                                    attT[(w, n // 4)][:, col : col + 1],
                                    nat_sb[k][:, n * 512 : (n + 1) * 512],
                                    start=(n == 0),
                                    stop=(n == caps[k] - 1),
                                )
                        pe_backlog.append(p3clos)
                    def fin(k=k, bwi=bwi, rvT=rvT, holder=holder):
                        os_b = os_pool.tile([1, H], F32, tag="os")
                        nc.vector.tensor_scalar_mul(
                            os_b[:], holder["ops"][:], rvT[0:1, bwi : bwi + 1]
                        )
                        nc.gpsimd.dma_start(out_d[k : k + 1, :], os_b[:])
                    pe_backlog.append(fin)

            for _rep in range(repeat):
                attT.clear()
                smxs.clear()
                for w, (b0, wb) in enumerate(WAVE_SPANS):
                    for bwi in range(wb):
                        trace_slot(w, b0, wb, bwi)
                    # softmax (DVE/ACT) fires immediately after the wave's
                    # last tanh; the PE part queues onto the backlog
                    finish_softmax(w, b0, wb)
                    push_finish_pe(w, b0, wb)
                while pe_backlog:
                    pe_backlog.popleft()()

    nc.finalize()
    return nc


_NC_CACHE = {}


def get_nc(caps, widths, repeat: int = 1) -> bass.Bass:
    key = (tuple(caps), tuple(widths), repeat)
    if key not in _NC_CACHE:
        _NC_CACHE[key] = build_nc(caps, widths, repeat=repeat)
    return _NC_CACHE[key]


def plan(lens):
    """Sort batches by length desc; slot k = ranks [8k, 8k+8), one per core.

    Window maxima of the descending sort minimize the summed per-slot caps
    (both the per-token p1/scores work and the 512*ceil(L/128) p3 term are
    monotone in L, so length-sorting is optimal for the combined cost).
    """
    lens = np.asarray(lens).reshape(B).astype(np.int64)
    order = np.argsort(-lens, kind="stable")
    assign = order.reshape(BPC, NCORES)          # [slot, core] -> batch
    widths = [int(lens[assign[k, 0]]) for k in range(BPC)]
    caps = [(w + 127) // 128 for w in widths]
    return caps, widths, assign


def make_in_maps(nn_outs, batch_lens, context, proj_w, proj_b, caps, widths,
                 assign):
    xf = np.asarray(nn_outs, dtype=np.float32)
    x_bf = xf.astype(ml_dtypes.bfloat16)
    x8 = (xf * SX).astype(E4M3)                  # [B, S, H] fp8
    lens = np.asarray(batch_lens).reshape(B).astype(np.int64)
    ctx_f = np.asarray(context, np.float32)
    Wf = np.asarray(proj_w, np.float32)          # [A, H]

    # --- W8 in SwInterleave ldweights layout, per (a-chunk, h-pair) ---
    W8 = (Wf * SW).astype(E4M3)
    # logical L[a,q][k,i,m] = W8[128a+m, 128(2q+i)+k]; storage st[k, 2j+i] =
    # L[k, i, 127-j] (pair-interleaved, columns reversed)
    Lf = np.ascontiguousarray(W8).reshape(AC, 128, 2, 2, 128)  # [a,m,q,i,k]
    T = Lf.transpose(0, 2, 4, 3, 1)                            # [a,q,k,i,m]
    st = T[..., ::-1].transpose(0, 1, 2, 4, 3)                 # [a,q,k,j,i]
    wt_host = np.ascontiguousarray(
        st.reshape(AC * 2, 128, 256).transpose(1, 0, 2).reshape(128, AC * 2 * 256)
    )

    # --- correction vector v = (W - W8)^T c, fp8, zero-padded stationary ---
    v = (Wf - W8.astype(np.float32) / SW).T @ ctx_f            # [H]
    v8 = (v * KV).astype(E4M3)
    vh = np.zeros((128, 2, WB, 2, VPAD), E4M3)
    vq = v8.reshape(2, 2, 128)                                 # [q, i, p]
    for bwi in range(WB):
        vh[:, :, bwi, :, bwi] = vq.transpose(2, 0, 1)
    v_host = np.ascontiguousarray(vh.reshape(128, 2 * WB * 2 * VPAD))

    # --- ctx (x SC) in the zero-padded wave layout ---
    ctx_c = (ctx_f * SC).reshape(AC, 128)
    ctx_host = np.zeros((128, AC, WB, VPAD), np.float32)
    for a in range(AC):
        for bw in range(WB):
            ctx_host[:, a, bw, bw] = ctx_c[a]
    ctx_host = np.ascontiguousarray(
        ctx_host.reshape(128, AC * WB * VPAD)
    ).astype(ml_dtypes.bfloat16)
    pb_host = np.ascontiguousarray(
        np.asarray(proj_b, np.float32).reshape(AC, 128).T
    )
    iota = np.arange(S)[None, :]

    xt_w = sum(4 * w for w in widths)
    nat_w = sum(c * 512 for c in caps)
    in_maps = []
    for c in range(NCORES):
        xt_all = np.empty((128, xt_w), E4M3)
        nat_all = np.empty((128, nat_w), ml_dtypes.bfloat16)
        mask = np.zeros((MW, len(WAVE_SPANS) * S), ml_dtypes.bfloat16)
        xo = no = 0
        for k in range(BPC):
            b = assign[k, c]
            W = widths[k]
            # xt8[p, q, i, s] = x8[b, s, 128(2q+i)+p]
            xt_all[:, xo : xo + 4 * W] = (
                x8[b, :W, :].T.reshape(2, 2, 128, W)
                .transpose(2, 0, 1, 3).reshape(128, 4 * W)
            )
            xo += 4 * W
            Wc = caps[k] * 128
            nat_all[:, no : no + caps[k] * 512] = (
                x_bf[b, :Wc, :].reshape(caps[k], 128, 512).transpose(1, 0, 2)
                .reshape(128, caps[k] * 512)
            )
            no += caps[k] * 512
        for w, (b0, wb) in enumerate(WAVE_SPANS):
            for bwi in range(wb):
                b = assign[b0 + bwi, c]
                mask[bwi, w * S : (w + 1) * S] = np.where(
                    iota[0] < lens[b], 0.0, -30000.0 * SC
                )
        in_maps.append(
            {
                "xt": xt_all,
                "nat": nat_all,
                "wt": wt_host,
                "ctx": ctx_host,
                "v8": v_host,
                "pb": pb_host,
                "mask": mask,
            }
        )
    return in_maps


def run(nn_outs, batch_lens, context, proj_w, proj_b, trace=False, repeat=1,
        **trace_kw):
    from concourse.bass_utils import run_bass_kernel_spmd

    caps, widths, assign = plan(batch_lens)
    nc = get_nc(caps, widths, repeat=repeat)
    in_maps = make_in_maps(
        nn_outs, batch_lens, context, proj_w, proj_b, caps, widths, assign
    )
    res = run_bass_kernel_spmd(
        nc, in_maps, list(range(NCORES)), trace=trace, **trace_kw
    )
    out = np.empty((B, H), np.float32)
    for c in range(NCORES):
        out[assign[:, c]] = res.results[c]["out"]
    return out, res


def kernel(nn_outs, batch_lens, context, proj_w, proj_b):
    out, _ = run(nn_outs, batch_lens, context, proj_w, proj_b, trace=False)
    return out


# revision 51
# speedup vs baseline: 1.0510x; 1.0007x over previous
"""Trainium2 Bass kernel for nn_AttenLayer (ragged-sequence attention pooling).

Math (per batch b, with length L_b):
    proj   = tanh(nn_outs @ W^T + b)           # (S, A)
    scores = proj @ context                     # (S,)
    atten  = masked_softmax(scores, L_b)        # (S,), zeros beyond L_b
    out    = atten @ nn_outs                    # (H,)

Ragged slot-capped data-parallel sharding over 8 cores (see plan()):
batches sorted by length desc; slot k = ranks [8k, 8k+8), one batch per
core per slot; all cores run one SPMD instruction stream sized by the
per-slot width caps.

Phase-1 runs in fp8(e4m3) dual-row mode (2x PE throughput, 2x contraction
per instruction = 4x fewer cycles than bf16): x is quantized to e4m3*16
in an h-pair-interleaved layout, W^T to e4m3*512 in the SwInterleave
ldweights layout.  The fp8 quantization error is tamed by a linear
correction: scores += alpha * v^T x8 with v = (W - W8)^T c precomputed on
host and alpha ~ E[tanh'] = 0.5, which cancels the W-quantization error in
the locally-linear regime of tanh (measured rel err 0.0168 vs 0.0217
uncorrected; gate is 2e-2).  The correction is a zero-padded-stationary
fp8 DoubleRow matvec (1 cycle/token) accumulated directly into the scores
PSUM; to keep its output scale representable in e4m3, the whole scores
pipeline (ctx, mask) is scaled by 4096 and descaled in the softmax exp.

Scores matmul stays bf16 (proj fp8 would add ~1.8% error), phase 3
(atten @ nn_outs) stays bf16 (nat fp8 would add ~2.5%).

The p1 phase is ACT(tanh)-bound, so scheduling centers on keeping the
tanh stream dense: waves of 1-2 slots run smallest-first (see WAVE_SPANS)
sharing a [64, 512]-per-chunk scores PSUM via the zero-padded stationary
trick; scores matmuls trail their phase-1 producers by PIPE a-steps
(software pipeline over the tanh latency); each wave's mask-add + max
runs per-chunk inline behind the psum stops; the wave's PE finish work
(ex transposes + phase-3 matmuls, ex transposed on PE as phase-3 lhsT,
1/sum applied on the final [1, H] copy) is deferred onto a backlog of
small closures consumed inside later slots' p1 loops, filling PE slack
without stalling the in-order PE queue behind softmax chains.
"""

import sys

for _p in ("/opt/trn_rl_repo",):
    if _p not in sys.path:
        sys.path.insert(0, _p)

import numpy as np
import ml_dtypes

import concourse.bass as bass
from concourse import bacc
import concourse.mybir as mybir
import concourse.tile as tile
from concourse.masks import make_identity

B, S, H, A = 64, 2048, 512, 512
NCORES = 8
BPC = B // NCORES          # slots per core
# wave processing order: small slots first so their phase-3 work is
# available early and fills the PE gaps of the big ACT-bound p1 phases;
# the biggest pair runs third so its large phase-3 block overlaps the
# following medium wave, and the tiniest slot ends the kernel.
WAVE_SPANS = [(5, 1), (2, 1), (0, 1), (1, 1), (3, 1), (4, 1), (6, 1), (7, 1)]
WB = 4                     # ctx zero-pad layout width (max columns)
MW = 2                     # max wave size (rows in masks/softmax tiles)

AC = A // 128              # 4 a-chunks
HC = H // 128              # 4 h-chunks (2 dual-row pairs)

F32 = mybir.dt.float32
BF16 = mybir.dt.bfloat16
FP8 = mybir.dt.float8e4
E4M3 = ml_dtypes.float8_e4m3
DR = mybir.MatmulPerfMode.DoubleRow
DRSW = mybir.MatmulPerfMode.DoubleRowSwInterleave

SX = 16.0                  # x fp8 scale
SW = 512.0                 # W fp8 scale
SC = 4096.0                # scores-psum (ctx/mask) scale
ALPHA = 0.5                # linearization coefficient ~ E[tanh'(y)]
KV = SC * ALPHA / SX       # v fp8 scale
PIPE = 2                   # scores closures trail p1 by this many (jt,a)-steps
VPAD = 64                  # dual-fp8 ldweights needs >=64 stationary cols


def build_nc(caps, widths, repeat: int = 1) -> bass.Bass:
    caps = [int(c) for c in caps]
    widths = [int(w) for w in widths]
    assert len(caps) == BPC and all(1 <= c <= S // 128 for c in caps)
    assert all(caps[i] >= caps[i + 1] for i in range(BPC - 1))
    assert all(widths[i] >= widths[i + 1] for i in range(BPC - 1))
    assert all((c - 1) * 128 < w <= c * 128 for c, w in zip(caps, widths))
    Wk = widths
    njs = [(w + 511) // 512 for w in Wk]     # 512-wide scores tiles
    xt_off = [0]
    nat_off = [0]
    for k in range(BPC):
        xt_off.append(xt_off[-1] + 4 * Wk[k])
        nat_off.append(nat_off[-1] + caps[k] * 512)

    nc = bacc.Bacc()

    xt_d = nc.declare_dram_parameter("xt", [128, xt_off[-1]], FP8, isOutput=False)
    nat_d = nc.declare_dram_parameter("nat", [128, nat_off[-1]], BF16, isOutput=False)
    # SwInterleave ldweights blocks per (a-chunk, h-pair): see make_in_maps
    wt_d = nc.declare_dram_parameter("wt", [128, AC * 2 * 256], FP8, isOutput=False)
    ctx_d = nc.declare_dram_parameter("ctx", [128, AC * WB * VPAD], BF16, isOutput=False)
    v_d = nc.declare_dram_parameter("v8", [128, 2 * WB * 2 * VPAD], FP8, isOutput=False)
    pb_d = nc.declare_dram_parameter("pb", [128, AC], F32, isOutput=False)
    # mask rows regrouped per wave so each wave's rows start at partition 0
    mask_d = nc.declare_dram_parameter(
        "mask", [MW, len(WAVE_SPANS) * S], BF16, isOutput=False
    )
    out_d = nc.declare_dram_parameter("out", [BPC, H], F32, isOutput=True)

    with tile.TileContext(nc) as tc:
        with (
            tc.tile_pool(name="const", bufs=1) as const_pool,
            tc.tile_pool(name="xt", bufs=3) as xt_pool,
            tc.tile_pool(name="nat", bufs=5) as nat_pool,
            tc.tile_pool(name="projT", bufs=6) as proj_pool,
            tc.tile_pool(name="smx", bufs=2) as smx_pool,
            tc.tile_pool(name="attT", bufs=8) as attT_pool,
            tc.tile_pool(name="osb", bufs=4) as os_pool,
            tc.tile_pool(name="p1ps", bufs=2, space="PSUM") as p1_psum,
            tc.tile_pool(name="scps", bufs=2, space="PSUM") as sc_psum,
            tc.tile_pool(name="atps", bufs=1, space="PSUM") as at_psum,
            tc.tile_pool(name="ops", bufs=1, space="PSUM") as out_psum,
        ):
            # ---- constants (DMAs deferred to first use in slot 0) ----
            wt_sb = const_pool.tile([128, AC * 2 * 256], FP8, tag="wt")
            ctx_sb = const_pool.tile([128, AC * WB * VPAD], BF16, tag="ctx")
            v_sb = const_pool.tile([128, 2, WB, 2, VPAD], FP8, tag="v8")
            pb_sb = const_pool.tile([128, AC], F32, tag="pb")
            ident = const_pool.tile([WB, WB], BF16, tag="ident")
            make_identity(nc, ident[:])
            nat_sb = {}

            mask_sb = const_pool.tile([MW, len(WAVE_SPANS) * S], BF16, tag="mask")
            attT = {}   # (w, g) -> [128, 4*MW] bf16, col = cs*jj + bwi
            smxs = {}   # w -> per-wave softmax tiles
            from collections import deque
            pe_backlog = deque()  # deferred transpose/p3 closures (PE work)
            consts_loaded = [False]

            def trace_slot(w, b0, wb, bwi):
                k = b0 + bwi
                W = Wk[k]
                xt = xt_pool.tile([128, 2, 2, 2048], FP8, tag="xt")
                src = xt_d[:, xt_off[k] : xt_off[k] + 4 * W].rearrange(
                    "p (q i w) -> p q i w", q=2, i=2
                )
                if not consts_loaded[0]:
                    consts_loaded[0] = True
                    # a=0's weight blocks first so the first p1 matmul can
                    # start as soon as the first xt lands
                    nc.sync.dma_start(wt_sb[:, :512], wt_d[:, :512])
                    nc.gpsimd.dma_start(pb_sb[:], pb_d[:])
                    nc.gpsimd.dma_start(
                        v_sb[:].rearrange("p a b c d -> p (a b c d)"), v_d[:]
                    )
                    nc.gpsimd.dma_start(ctx_sb[:], ctx_d[:])
                if bwi == 0:
                    # this wave's mask rows, just in time for its softmax
                    nc.gpsimd.dma_start(
                        mask_sb[:, w * S : (w + 1) * S],
                        mask_d[:, w * S : (w + 1) * S],
                    )
                if w <= 1 and W > 1024:
                    # early big slots: land the first p1 chunks fast
                    for c0 in range(0, W, 1024):
                        c1 = min(W, c0 + 1024)
                        nc.sync.dma_start(
                            xt[:, :, :, c0:c1], src[:, :, :, c0:c1]
                        )
                else:
                    nc.sync.dma_start(xt[:, :, :, :W], src[:])
                if bwi == 0 and w == 0:
                    nc.sync.dma_start(wt_sb[:, 512:], wt_d[:, 512:])
                natk = nat_pool.tile([128, 16 * 512], BF16, tag="nat")
                nat_sb[k] = natk
                # nat is phase-3 data (needed late): keep it all on the Pool
                # queue so xt flows without queueing delay on SP
                nc.gpsimd.dma_start(
                    natk[:, : caps[k] * 512],
                    nat_d[:, nat_off[k] : nat_off[k + 1]],
                )
                # per-slot 512-wide score psum tiles (2-bank ring): each slot
                # drains its own row into the wave's shared scm right after
                # its ctx stops, so only the exp chain remains per wave.
                # the wave's scm/pmax SBUF tiles are shared across its slots.
                if bwi == 0:
                    smxs[w] = {
                        "scm": smx_pool.tile([MW, S], F32, tag="scm", name="scm"),
                        "pmax": smx_pool.tile([MW, 4], F32, tag="pmax", name="pmax"),
                    }
                    nc.vector.memset(smxs[w]["pmax"][:], -3.0e38)
                    # narrow rows' scm tails up to the wave width would be
                    # uninitialized: kill them now (row 0's drains overwrite
                    # its real region later)
                    if wb > 1 and Wk[b0] > Wk[b0 + wb - 1]:
                        nc.vector.memset(
                            smxs[w]["scm"][:wb, Wk[b0 + wb - 1] : Wk[b0]],
                            -3.0e9 * SC,
                        )
                scm = smxs[w]["scm"]
                pmax = smxs[w]["pmax"]
                scd = {}      # j512 -> this slot's score psum tile

                def chunk_fin(j):
                    wjj = min(512, W - j * 512)
                    sl = slice(j * 512, j * 512 + wjj)
                    nc.vector.tensor_tensor(
                        out=scm[bwi : bwi + 1, sl],
                        in0=scd[j][bwi : bwi + 1, :wjj],
                        in1=mask_sb[bwi : bwi + 1,
                                    w * S + j * 512 : w * S + j * 512 + wjj],
                        op=mybir.AluOpType.add,
                    )
                    nc.vector.reduce_max(
                        pmax[bwi : bwi + 1, j : j + 1],
                        scm[bwi : bwi + 1, sl],
                        axis=mybir.AxisListType.X,
                    )

                NJ4 = (W + 1023) // 1024
                pq = []  # scores closures trail p1 by PIPE (jt,a)-steps
                for jt in range(NJ4):
                    b4 = jt * 1024
                    wj4 = min(1024, W - b4)
                    # 512-wide regions of the 2-bank p1 psum tile
                    regions = [(0, min(512, wj4))]
                    if wj4 > 512:
                        regions.append((512, wj4 - 512))
                    for a in range(AC):
                        if len(pq) >= PIPE:
                            pq.pop(0)()
                        if a == 1 and jt > 0:
                            # previous iter's chunks: their stops popped at
                            # this step; drain now, one step before their
                            # psum ring slots get reused
                            for j in range(2 * (jt - 1), 2 * jt):
                                if j in scd:
                                    chunk_fin(j)
                        ps = p1_psum.tile([128, 1024], F32, tag="p1")
                        for ro, rw in regions:
                            nq = (rw + 255) // 256
                            for qi in range(nq):
                                qo = ro + qi * 256
                                qw = min(256, rw - qi * 256)
                                for q in range(2):
                                    nc.tensor.matmul(
                                        ps[:, qo : qo + qw],
                                        wt_sb[:, (a * 2 + q) * 256 : (a * 2 + q + 1) * 256],
                                        xt[:, q, :, b4 + qo : b4 + qo + qw],
                                        start=(qi == 0 and q == 0),
                                        stop=(qi == nq - 1 and q == 1),
                                        perf_mode=DRSW,
                                    )
                        pt = proj_pool.tile([128, 1024], BF16, tag="projT")
                        nc.scalar.activation(
                            pt[:, :wj4],
                            ps[:, :wj4],
                            mybir.ActivationFunctionType.Tanh,
                            bias=pb_sb[:, a : a + 1],
                            scale=1.0 / (SX * SW),
                        )
                        # ctx col bwi is context's a-chunk (x SC), others
                        # zero, padded to VPAD cols to match the v matmuls'
                        # psum partitions.  a==0 allocates+starts the score
                        # tiles; a==1 adds the fp8 v-correction matvecs.
                        def pend(pt=pt, b4=b4, a=a, regions=regions, xt=xt):
                            for ro, rw in regions:
                                j = (b4 + ro) // 512
                                if a == 0:
                                    scd[j] = sc_psum.tile(
                                        [VPAD, 512], F32, tag="scps",
                                        name="scps",
                                    )
                                nc.tensor.matmul(
                                    scd[j][:, :rw],
                                    ctx_sb[:, (a * WB + bwi) * VPAD : (a * WB + bwi + 1) * VPAD],
                                    pt[:, ro : ro + rw],
                                    start=(a == 0),
                                    stop=(a == AC - 1),
                                )
                                if a == 1:
                                    for qo in range(0, rw, 256):
                                        qw = min(256, rw - qo)
                                        for q in range(2):
                                            nc.tensor.matmul(
                                                scd[j][:, qo : qo + qw],
                                                v_sb[:, q, bwi, :, :],
                                                xt[:, q, :, b4 + ro + qo : b4 + ro + qo + qw],
                                                start=False,
                                                stop=False,
                                                perf_mode=DR,
                                            )
                        pq.append(pend)
                        # fill PE slack with deferred transpose/p3 closures
                        if pe_backlog and (a % 2 == 1 or len(pe_backlog) > 12):
                            pe_backlog.popleft()()
                for f in pq:
                    f()
                for j in range(max(0, 2 * (NJ4 - 1)), njs[k]):
                    if j in scd:
                        chunk_fin(j)


            def finish_softmax(w, b0, wb):
                Wmax = Wk[b0]
                njw = njs[b0]
                capm = caps[b0]
                # per-chunk masked scores + maxes were computed inline by the
                # wave's last trace_slot; only the exp chain remains here
                scm = smxs[w]["scm"]
                pmax = smxs[w]["pmax"]
                mx = smx_pool.tile([wb, 1], F32, tag="mx")
                nc.vector.reduce_max(
                    mx[:], pmax[:wb, :njw], axis=mybir.AxisListType.X, negate=True
                )
                mx2 = smx_pool.tile([wb, 1], F32, tag="mx2")
                nc.vector.tensor_scalar_mul(mx2[:], mx[:], 1.0 / SC)
                ex = smx_pool.tile([MW, S], BF16, tag="ex")
                rs = smx_pool.tile([wb, 1], F32, tag="rs")
                nc.scalar.activation(
                    ex[:wb, :Wmax],
                    scm[:wb, :Wmax],
                    mybir.ActivationFunctionType.Exp,
                    bias=mx2[:],
                    scale=1.0 / SC,
                    accum_out=rs[:],
                )
                rv = smx_pool.tile([wb, 1], F32, tag="rv")
                nc.vector.reciprocal(rv[:], rs[:])
                if wb > 1:
                    # move rv to partition 0 (tiny SBUF->SBUF DMA) so the
                    # final output copy can read it as a per-partition scale
                    rvT = smx_pool.tile([1, MW], F32, tag="rvT")
                    nc.gpsimd.dma_start(rvT[0:1, :wb], rv[:, 0:1])
                else:
                    rvT = rv  # single-row wave: already at partition 0
                # zero the ex tail beyond the wave width so the last
                # 128-chunk transposes clean zeros
                if capm * 128 > Wmax:
                    nc.vector.memset(ex[:wb, Wmax : capm * 128], 0.0)
                smxs[w]["ex"] = ex
                smxs[w]["rvT"] = rvT

            def push_finish_pe(w, b0, wb):
                """Queue the wave's PE work (ex transposes + phase 3) as
                small closures consumed inside later slots' p1 loops."""
                capm = caps[b0]
                ex, rvT = smxs[w]["ex"], smxs[w]["rvT"]
                # column stride even so bf16 PSUM offsets stay 4B-aligned
                cs = 2
                for g in range((capm + 3) // 4):
                    def tclos(g=g, wb=wb, ex=ex, w=w):
                        na = min(4, capm - 4 * g)
                        aps = at_psum.tile([128, 4 * MW], BF16, tag="atps")
                        for jj in range(na):
                            ch = 4 * g + jj
                            nc.tensor.transpose(
                                aps[:, jj * cs : jj * cs + wb],
                                ex[:wb, ch * 128 : (ch + 1) * 128],
                                ident[:wb, :wb],
                            )
                        att_sb = attT_pool.tile([128, 4 * MW], BF16, tag="attT")
                        nc.vector.tensor_copy(
                            att_sb[:, : na * cs].rearrange(
                                "p (n c) -> p n c", c=cs
                            )[:, :, :wb],
                            aps[:, : na * cs].rearrange(
                                "p (n c) -> p n c", c=cs
                            )[:, :, :wb],
                        )
                        attT[(w, g)] = att_sb
                    pe_backlog.append(tclos)
                # phase 3: out[k] = (sum_s ex[s] * x[s, :]) * rv
                for bwi in reversed(range(wb)):
                    k = b0 + bwi
                    holder = {}
                    for n0 in range(0, caps[k], 2):
                        def p3clos(n0=n0, k=k, bwi=bwi, w=w, holder=holder):
                            if n0 == 0:
                                holder["ops"] = out_psum.tile(
                                    [1, H], F32, tag="ops", name="ops"
                                )
                            ops = holder["ops"]
                            for n in range(n0, min(n0 + 2, caps[k])):
                                col = (n % 4) * cs + bwi
                                nc.tensor.matmul(
                                    ops[:],
                                    attT[(w, n // 4)][:, col : col + 1],
                                    nat_sb[k][:, n * 512 : (n + 1) * 512],
                                    start=(n == 0),
                                    stop=(n == caps[k] - 1),
                                )
                        pe_backlog.append(p3clos)
                    def fin(k=k, bwi=bwi, rvT=rvT, holder=holder):
                        os_b = os_pool.tile([1, H], F32, tag="os")
                        nc.vector.tensor_scalar_mul(
                            os_b[:], holder["ops"][:], rvT[0:1, bwi : bwi + 1]
                        )
                        nc.gpsimd.dma_start(out_d[k : k + 1, :], os_b[:])
                    pe_backlog.append(fin)

            for _rep in range(repeat):
                attT.clear()
                smxs.clear()
                for w, (b0, wb) in enumerate(WAVE_SPANS):
                    for bwi in range(wb):
                        trace_slot(w, b0, wb, bwi)
                    # softmax (DVE/ACT) fires immediately after the wave's
                    # last tanh; the PE part queues onto the backlog
                    finish_softmax(w, b0, wb)
                    push_finish_pe(w, b0, wb)
                while pe_backlog:
                    pe_backlog.popleft()()

    nc.finalize()
    return nc


_NC_CACHE = {}


def get_nc(caps, widths, repeat: int = 1) -> bass.Bass:
    key = (tuple(caps), tuple(widths), repeat)
    if key not in _NC_CACHE:
        _NC_CACHE[key] = build_nc(caps, widths, repeat=repeat)
    return _NC_CACHE[key]


def plan(lens):
    """Sort batches by length desc; slot k = ranks [8k, 8k+8), one per core.

    Window maxima of the descending sort minimize the summed per-slot caps
    (both the per-token p1/scores work and the 512*ceil(L/128) p3 term are
    monotone in L, so length-sorting is optimal for the combined cost).
    """
    lens = np.asarray(lens).reshape(B).astype(np.int64)
    order = np.argsort(-lens, kind="stable")
    assign = order.reshape(BPC, NCORES)          # [slot, core] -> batch
    widths = [int(lens[assign[k, 0]]) for k in range(BPC)]
    caps = [(w + 127) // 128 for w in widths]
    return caps, widths, assign


def make_in_maps(nn_outs, batch_lens, context, proj_w, proj_b, caps, widths,
                 assign):
    xf = np.asarray(nn_outs, dtype=np.float32)
    x_bf = xf.astype(ml_dtypes.bfloat16)
    x8 = (xf * SX).astype(E4M3)                  # [B, S, H] fp8
    lens = np.asarray(batch_lens).reshape(B).astype(np.int64)
    ctx_f = np.asarray(context, np.float32)
    Wf = np.asarray(proj_w, np.float32)          # [A, H]

    # --- W8 in SwInterleave ldweights layout, per (a-chunk, h-pair) ---
    W8 = (Wf * SW).astype(E4M3)
    # logical L[a,q][k,i,m] = W8[128a+m, 128(2q+i)+k]; storage st[k, 2j+i] =
    # L[k, i, 127-j] (pair-interleaved, columns reversed)
    Lf = np.ascontiguousarray(W8).reshape(AC, 128, 2, 2, 128)  # [a,m,q,i,k]
    T = Lf.transpose(0, 2, 4, 3, 1)                            # [a,q,k,i,m]
    st = T[..., ::-1].transpose(0, 1, 2, 4, 3)                 # [a,q,k,j,i]
    wt_host = np.ascontiguousarray(
        st.reshape(AC * 2, 128, 256).transpose(1, 0, 2).reshape(128, AC * 2 * 256)
    )

    # --- correction vector v = (W - W8)^T c, fp8, zero-padded stationary ---
    v = (Wf - W8.astype(np.float32) / SW).T @ ctx_f            # [H]
    v8 = (v * KV).astype(E4M3)
    vh = np.zeros((128, 2, WB, 2, VPAD), E4M3)
    vq = v8.reshape(2, 2, 128)                                 # [q, i, p]
    for bwi in range(WB):
        vh[:, :, bwi, :, bwi] = vq.transpose(2, 0, 1)
    v_host = np.ascontiguousarray(vh.reshape(128, 2 * WB * 2 * VPAD))

    # --- ctx (x SC) in the zero-padded wave layout ---
    ctx_c = (ctx_f * SC).reshape(AC, 128)
    ctx_host = np.zeros((128, AC, WB, VPAD), np.float32)
    for a in range(AC):
        for bw in range(WB):
            ctx_host[:, a, bw, bw] = ctx_c[a]
    ctx_host = np.ascontiguousarray(
        ctx_host.reshape(128, AC * WB * VPAD)
    ).astype(ml_dtypes.bfloat16)
    pb_host = np.ascontiguousarray(
        np.asarray(proj_b, np.float32).reshape(AC, 128).T
    )
    iota = np.arange(S)[None, :]

    xt_w = sum(4 * w for w in widths)
    nat_w = sum(c * 512 for c in caps)
    in_maps = []
    for c in range(NCORES):
        xt_all = np.empty((128, xt_w), E4M3)
        nat_all = np.empty((128, nat_w), ml_dtypes.bfloat16)
        mask = np.zeros((MW, len(WAVE_SPANS) * S), ml_dtypes.bfloat16)
        xo = no = 0
        for k in range(BPC):
            b = assign[k, c]
            W = widths[k]
            # xt8[p, q, i, s] = x8[b, s, 128(2q+i)+p]
            xt_all[:, xo : xo + 4 * W] = (
                x8[b, :W, :].T.reshape(2, 2, 128, W)
                .transpose(2, 0, 1, 3).reshape(128, 4 * W)
            )
            xo += 4 * W
            Wc = caps[k] * 128
            nat_all[:, no : no + caps[k] * 512] = (
                x_bf[b, :Wc, :].reshape(caps[k], 128, 512).transpose(1, 0, 2)
                .reshape(128, caps[k] * 512)
            )
            no += caps[k] * 512
        for w, (b0, wb) in enumerate(WAVE_SPANS):
            for bwi in range(wb):
                b = assign[b0 + bwi, c]
                mask[bwi, w * S : (w + 1) * S] = np.where(
                    iota[0] < lens[b], 0.0, -30000.0 * SC
                )
        in_maps.append(
            {
                "xt": xt_all,
                "nat": nat_all,
                "wt": wt_host,
                "ctx": ctx_host,
                "v8": v_host,
                "pb": pb_host,
                "mask": mask,
            }
        )
    return in_maps


def run(nn_outs, batch_lens, context, proj_w, proj_b, trace=False, repeat=1,
        **trace_kw):
    from concourse.bass_utils import run_bass_kernel_spmd

    caps, widths, assign = plan(batch_lens)
    nc = get_nc(caps, widths, repeat=repeat)
    in_maps = make_in_maps(
        nn_outs, batch_lens, context, proj_w, proj_b, caps, widths, assign
    )
    res = run_bass_kernel_spmd(
        nc, in_maps, list(range(NCORES)), trace=trace, **trace_kw
    )
    out = np.empty((B, H), np.float32)
    for c in range(NCORES):
        out[assign[:, c]] = res.results[c]["out"]
    return out, res


def kernel(nn_outs, batch_lens, context, proj_w, proj_b):
    out, _ = run(nn_outs, batch_lens, context, proj_w, proj_b, trace=False)
    return out


# revision 68
# speedup vs baseline: 1.0766x; 1.0244x over previous
"""Trainium2 Bass kernel for nn_AttenLayer (ragged-sequence attention pooling).

Math (per batch b, with length L_b):
    proj   = tanh(nn_outs @ W^T + b)           # (S, A)
    scores = proj @ context                     # (S,)
    atten  = masked_softmax(scores, L_b)        # (S,), zeros beyond L_b
    out    = atten @ nn_outs                    # (H,)

Ragged slot-capped data-parallel sharding over 8 cores (see plan()):
batches sorted by length desc; slot k = ranks [8k, 8k+8), one batch per
core per slot; all cores run one SPMD instruction stream sized by the
per-slot width caps.

Phase-1 runs in fp8(e4m3) dual-row mode (2x PE throughput, 2x contraction
per instruction = 4x fewer cycles than bf16): x is quantized to e4m3*16
in an h-pair-interleaved layout, W^T to e4m3*512 in the SwInterleave
ldweights layout.  The fp8 quantization error is tamed by a linear
correction: scores += alpha * v^T x8 with v = (W - W8)^T c precomputed on
host and alpha ~ E[tanh'] = 0.5, which cancels the W-quantization error in
the locally-linear regime of tanh (measured rel err 0.0168 vs 0.0217
uncorrected; gate is 2e-2).  The correction is a zero-padded-stationary
fp8 DoubleRow matvec (1 cycle/token) accumulated directly into the scores
PSUM; to keep its output scale representable in e4m3, the whole scores
pipeline (ctx, mask) is scaled by 4096 and descaled in the softmax exp.

Scores matmul stays bf16 (proj fp8 would add ~1.8% error), phase 3
(atten @ nn_outs) stays bf16 (nat fp8 would add ~2.5%).

PE and ACT are co-critical (~50us each), so the structure keeps both
dense: slots are processed as singleton waves (WAVE_SPANS order: medium
first for warmup, giants early-middle so their phase-3 fills later PE
slack, tiny last for a short tail).  Phase-1 uses [128,1024] 2-bank psum
tiles with 1024-wide tanh (halves ACT instruction overhead); each slot's
scores accumulate in per-slot [64,512] psum tiles on a 2-bank ring,
drained (mask-add + max, DVE) into SBUF one pipeline step after their ctx
stops -- the closure pipeline (pq, PIPE steps deep) orders pop -> drain ->
alloc/ctx-start(a==0) -> v-correction(a==1) so the ring never stalls the
in-order PE queue.  Each slot then runs its own exp chain; the PE finish
work (ex transposes + phase-3 matmuls, ex transposed on PE as phase-3
lhsT, 1/sum applied on the final [1, H] copy) is deferred onto a backlog
of small closures consumed inside later slots' p1 loops.
"""

import sys

for _p in ("/opt/trn_rl_repo",):
    if _p not in sys.path:
        sys.path.insert(0, _p)

import numpy as np
import ml_dtypes

import concourse.bass as bass
from concourse import bacc
import concourse.mybir as mybir
import concourse.tile as tile
from concourse.masks import make_identity

B, S, H, A = 64, 2048, 512, 512
NCORES = 8
BPC = B // NCORES          # slots per core
# wave processing order: small slots first so their phase-3 work is
# available early and fills the PE gaps of the big ACT-bound p1 phases;
# the biggest pair runs third so its large phase-3 block overlaps the
# following medium wave, and the tiniest slot ends the kernel.
WAVE_SPANS = [(4, 1), (2, 1), (0, 1), (1, 1), (3, 1), (5, 1), (6, 1), (7, 1)]
WB = 4                     # ctx zero-pad layout width (max columns)
MW = 2                     # max wave size (rows in masks/softmax tiles)

AC = A // 128              # 4 a-chunks
HC = H // 128              # 4 h-chunks (2 dual-row pairs)

F32 = mybir.dt.float32
BF16 = mybir.dt.bfloat16
FP8 = mybir.dt.float8e4
E4M3 = ml_dtypes.float8_e4m3
DR = mybir.MatmulPerfMode.DoubleRow
DRSW = mybir.MatmulPerfMode.DoubleRowSwInterleave

SX = 16.0                  # x fp8 scale
SW = 512.0                 # W fp8 scale
SC = 4096.0                # scores-psum (ctx/mask) scale
ALPHA = 0.5                # linearization coefficient ~ E[tanh'(y)]
KV = SC * ALPHA / SX       # v fp8 scale
PIPE = 2                   # scores closures trail p1 by this many (jt,a)-steps
VPAD = 64                  # dual-fp8 ldweights needs >=64 stationary cols


def build_nc(caps, widths, repeat: int = 1) -> bass.Bass:
    caps = [int(c) for c in caps]
    widths = [int(w) for w in widths]
    assert len(caps) == BPC and all(1 <= c <= S // 128 for c in caps)
    assert all(caps[i] >= caps[i + 1] for i in range(BPC - 1))
    assert all(widths[i] >= widths[i + 1] for i in range(BPC - 1))
    assert all((c - 1) * 128 < w <= c * 128 for c, w in zip(caps, widths))
    Wk = widths
    njs = [(w + 511) // 512 for w in Wk]     # 512-wide scores tiles
    xt_off = [0]
    nat_off = [0]
    for k in range(BPC):
        xt_off.append(xt_off[-1] + 4 * Wk[k])
        nat_off.append(nat_off[-1] + caps[k] * 512)

    nc = bacc.Bacc()

    xt_d = nc.declare_dram_parameter("xt", [128, xt_off[-1]], FP8, isOutput=False)
    nat_d = nc.declare_dram_parameter("nat", [128, nat_off[-1]], BF16, isOutput=False)
    # SwInterleave ldweights blocks per (a-chunk, h-pair): see make_in_maps
    wt_d = nc.declare_dram_parameter("wt", [128, AC * 2 * 256], FP8, isOutput=False)
    ctx_d = nc.declare_dram_parameter("ctx", [128, AC * WB * VPAD], BF16, isOutput=False)
    v_d = nc.declare_dram_parameter("v8", [128, 2 * WB * 2 * VPAD], FP8, isOutput=False)
    pb_d = nc.declare_dram_parameter("pb", [128, AC], F32, isOutput=False)
    # mask rows regrouped per wave so each wave's rows start at partition 0
    mask_d = nc.declare_dram_parameter(
        "mask", [MW, len(WAVE_SPANS) * S], BF16, isOutput=False
    )
    out_d = nc.declare_dram_parameter("out", [BPC, H], F32, isOutput=True)

    with tile.TileContext(nc) as tc:
        with (
            tc.tile_pool(name="const", bufs=1) as const_pool,
            tc.tile_pool(name="xt", bufs=3) as xt_pool,
            tc.tile_pool(name="nat", bufs=5) as nat_pool,
            tc.tile_pool(name="projT", bufs=6) as proj_pool,
            tc.tile_pool(name="smx", bufs=2) as smx_pool,
            tc.tile_pool(name="attT", bufs=8) as attT_pool,
            tc.tile_pool(name="osb", bufs=4) as os_pool,
            tc.tile_pool(name="p1ps", bufs=2, space="PSUM") as p1_psum,
            tc.tile_pool(name="scps", bufs=2, space="PSUM") as sc_psum,
            tc.tile_pool(name="atps", bufs=2, space="PSUM") as at_psum,
        ):
            # ---- constants (DMAs deferred to first use in slot 0) ----
            wt_sb = const_pool.tile([128, AC * 2 * 256], FP8, tag="wt")
            ctx_sb = const_pool.tile([128, AC * WB * VPAD], BF16, tag="ctx")
            v_sb = const_pool.tile([128, 2, WB, 2, VPAD], FP8, tag="v8")
            pb_sb = const_pool.tile([128, AC], F32, tag="pb")
            ident = const_pool.tile([WB, WB], BF16, tag="ident")
            make_identity(nc, ident[:])
            nat_sb = {}

            mask_sb = const_pool.tile([MW, len(WAVE_SPANS) * S], BF16, tag="mask")
            attT = {}   # (w, g) -> [128, 4*MW] bf16, col = cs*jj + bwi
            smxs = {}   # w -> per-wave softmax tiles
            from collections import deque
            pe_backlog = deque()  # deferred transpose/p3 closures (PE work)
            consts_loaded = [False]

            def trace_slot(w, b0, wb, bwi):
                k = b0 + bwi
                W = Wk[k]
                xt = xt_pool.tile([128, 2, 2, 2048], FP8, tag="xt")
                src = xt_d[:, xt_off[k] : xt_off[k] + 4 * W].rearrange(
                    "p (q i w) -> p q i w", q=2, i=2
                )
                if not consts_loaded[0]:
                    consts_loaded[0] = True
                    # a=0's weight blocks first so the first p1 matmul can
                    # start as soon as the first xt lands
                    nc.sync.dma_start(wt_sb[:, :512], wt_d[:, :512])
                    nc.gpsimd.dma_start(pb_sb[:], pb_d[:])
                    nc.gpsimd.dma_start(
                        v_sb[:].rearrange("p a b c d -> p (a b c d)"), v_d[:]
                    )
                    nc.gpsimd.dma_start(ctx_sb[:], ctx_d[:])
                if w <= 1 and W > 1024:
                    # early big slots: land the first p1 chunks fast
                    for c0 in range(0, W, 1024):
                        c1 = min(W, c0 + 1024)
                        nc.sync.dma_start(
                            xt[:, :, :, c0:c1], src[:, :, :, c0:c1]
                        )
                else:
                    nc.sync.dma_start(xt[:, :, :, :W], src[:])
                if bwi == 0 and w == 0:
                    nc.sync.dma_start(wt_sb[:, 512:], wt_d[:, 512:])
                # this wave's mask rows after xt (needed later, by the drains)
                nc.sync.dma_start(
                    mask_sb[:, w * S : (w + 1) * S],
                    mask_d[:, w * S : (w + 1) * S],
                )
                natk = nat_pool.tile([128, 16 * 512], BF16, tag="nat")
                nat_sb[k] = natk
                # nat is phase-3 data (needed late): keep it all on the Pool
                # queue so xt flows without queueing delay on SP
                nc.gpsimd.dma_start(
                    natk[:, : caps[k] * 512],
                    nat_d[:, nat_off[k] : nat_off[k + 1]],
                )
                # per-slot 512-wide score psum tiles (2-bank ring): each slot
                # drains its own row into the wave's shared scm right after
                # its ctx stops, so only the exp chain remains per wave.
                # the wave's scm/pmax SBUF tiles are shared across its slots.
                if bwi == 0:
                    smxs[w] = {
                        "scm": smx_pool.tile([MW, S], F32, tag="scm", name="scm"),
                        "pmax": smx_pool.tile([MW, 4], F32, tag="pmax", name="pmax"),
                    }
                    nc.vector.memset(smxs[w]["pmax"][:], -3.0e38)
                    # narrow rows' scm tails up to the wave width would be
                    # uninitialized: kill them now (row 0's drains overwrite
                    # its real region later)
                    if wb > 1 and Wk[b0] > Wk[b0 + wb - 1]:
                        nc.vector.memset(
                            smxs[w]["scm"][:wb, Wk[b0 + wb - 1] : Wk[b0]],
                            -3.0e9 * SC,
                        )
                scm = smxs[w]["scm"]
                pmax = smxs[w]["pmax"]
                scd = {}      # j512 -> this slot's score psum tile

                def chunk_fin(j):
                    wjj = min(512, W - j * 512)
                    sl = slice(j * 512, j * 512 + wjj)
                    nc.vector.tensor_tensor(
                        out=scm[bwi : bwi + 1, sl],
                        in0=scd[j][bwi : bwi + 1, :wjj],
                        in1=mask_sb[bwi : bwi + 1,
                                    w * S + j * 512 : w * S + j * 512 + wjj],
                        op=mybir.AluOpType.add,
                    )
                    nc.vector.reduce_max(
                        pmax[bwi : bwi + 1, j : j + 1],
                        scm[bwi : bwi + 1, sl],
                        axis=mybir.AxisListType.X,
                    )

                NJ4 = (W + 1023) // 1024
                pq = []  # scores closures trail p1 by PIPE (jt,a)-steps
                for jt in range(NJ4):
                    b4 = jt * 1024
                    wj4 = min(1024, W - b4)
                    # 512-wide regions of the 2-bank p1 psum tile
                    regions = [(0, min(512, wj4))]
                    if wj4 > 512:
                        regions.append((512, wj4 - 512))
                    for a in range(AC):
                        if len(pq) >= PIPE:
                            pq.pop(0)()
                        if a == 1 and jt > 0:
                            # previous iter's chunks: their stops popped at
                            # this step; drain now, one step before their
                            # psum ring slots get reused
                            for j in range(2 * (jt - 1), 2 * jt):
                                if j in scd:
                                    chunk_fin(j)
                        ps = p1_psum.tile([128, 1024], F32, tag="p1")
                        for ro, rw in regions:
                            nq = (rw + 255) // 256
                            for qi in range(nq):
                                qo = ro + qi * 256
                                qw = min(256, rw - qi * 256)
                                for q in range(2):
                                    nc.tensor.matmul(
                                        ps[:, qo : qo + qw],
                                        wt_sb[:, (a * 2 + q) * 256 : (a * 2 + q + 1) * 256],
                                        xt[:, q, :, b4 + qo : b4 + qo + qw],
                                        start=(qi == 0 and q == 0),
                                        stop=(qi == nq - 1 and q == 1),
                                        perf_mode=DRSW,
                                    )
                        pt = proj_pool.tile([128, 1024], BF16, tag="projT")
                        nc.scalar.activation(
                            pt[:, :wj4],
                            ps[:, :wj4],
                            mybir.ActivationFunctionType.Tanh,
                            bias=pb_sb[:, a : a + 1],
                            scale=1.0 / (SX * SW),
                        )
                        # ctx col bwi is context's a-chunk (x SC), others
                        # zero, padded to VPAD cols to match the v matmuls'
                        # psum partitions.  a==0 allocates+starts the score
                        # tiles; a==1 adds the fp8 v-correction matvecs.
                        def pend(pt=pt, b4=b4, a=a, regions=regions, xt=xt):
                            for ro, rw in regions:
                                j = (b4 + ro) // 512
                                if a == 0:
                                    scd[j] = sc_psum.tile(
                                        [VPAD, 512], F32, tag="scps",
                                        name="scps",
                                    )
                                nc.tensor.matmul(
                                    scd[j][:, :rw],
                                    ctx_sb[:, (a * WB + bwi) * VPAD : (a * WB + bwi + 1) * VPAD],
                                    pt[:, ro : ro + rw],
                                    start=(a == 0),
                                    stop=(a == AC - 1),
                                )
                                if a == 1:
                                    for qo in range(0, rw, 256):
                                        qw = min(256, rw - qo)
                                        for q in range(2):
                                            nc.tensor.matmul(
                                                scd[j][:, qo : qo + qw],
                                                v_sb[:, q, bwi, :, :],
                                                xt[:, q, :, b4 + ro + qo : b4 + ro + qo + qw],
                                                start=False,
                                                stop=False,
                                                perf_mode=DR,
                                            )
                        pq.append(pend)
                        # fill PE slack with deferred transpose/p3 closures
                        if pe_backlog and (a % 2 == 1 or len(pe_backlog) > 6):
                            pe_backlog.popleft()()
                for f in pq:
                    f()
                for j in range(max(0, 2 * (NJ4 - 1)), njs[k]):
                    if j in scd:
                        chunk_fin(j)


            def finish_softmax(w, b0, wb):
                Wmax = Wk[b0]
                njw = njs[b0]
                capm = caps[b0]
                # per-chunk masked scores + maxes were computed inline by the
                # wave's last trace_slot; only the exp chain remains here
                scm = smxs[w]["scm"]
                pmax = smxs[w]["pmax"]
                mx = smx_pool.tile([wb, 1], F32, tag="mx")
                nc.vector.reduce_max(
                    mx[:], pmax[:wb, :njw], axis=mybir.AxisListType.X, negate=True
                )
                mx2 = smx_pool.tile([wb, 1], F32, tag="mx2")
                nc.vector.tensor_scalar_mul(mx2[:], mx[:], 1.0 / SC)
                ex = smx_pool.tile([MW, S], BF16, tag="ex")
                rs = smx_pool.tile([wb, 1], F32, tag="rs")
                nc.scalar.activation(
                    ex[:wb, :Wmax],
                    scm[:wb, :Wmax],
                    mybir.ActivationFunctionType.Exp,
                    bias=mx2[:],
                    scale=1.0 / SC,
                    accum_out=rs[:],
                )
                rv = smx_pool.tile([wb, 1], F32, tag="rv")
                nc.vector.reciprocal(rv[:], rs[:])
                if wb > 1:
                    # move rv to partition 0 (tiny SBUF->SBUF DMA) so the
                    # final output copy can read it as a per-partition scale
                    rvT = smx_pool.tile([1, MW], F32, tag="rvT")
                    nc.gpsimd.dma_start(rvT[0:1, :wb], rv[:, 0:1])
                else:
                    rvT = rv  # single-row wave: already at partition 0
                # zero the ex tail beyond the wave width so the last
                # 128-chunk transposes clean zeros
                if capm * 128 > Wmax:
                    nc.vector.memset(ex[:wb, Wmax : capm * 128], 0.0)
                smxs[w]["ex"] = ex
                smxs[w]["rvT"] = rvT

            def push_finish_pe(w, b0, wb):
                """Queue the wave's PE work (ex transposes + phase 3) as
                small closures consumed inside later slots' p1 loops."""
                capm = caps[b0]
                ex, rvT = smxs[w]["ex"], smxs[w]["rvT"]
                # column stride even so bf16 PSUM offsets stay 4B-aligned
                cs = 2
                for g in range((capm + 3) // 4):
                    def tclos(g=g, wb=wb, ex=ex, w=w):
                        na = min(4, capm - 4 * g)
                        aps = at_psum.tile([128, 4 * MW], BF16, tag="atps")
                        for jj in range(na):
                            ch = 4 * g + jj
                            nc.tensor.transpose(
                                aps[:, jj * cs : jj * cs + wb],
                                ex[:wb, ch * 128 : (ch + 1) * 128],
                                ident[:wb, :wb],
                            )
                        att_sb = attT_pool.tile([128, 4 * MW], BF16, tag="attT")
                        nc.vector.tensor_copy(
                            att_sb[:, : na * cs].rearrange(
                                "p (n c) -> p n c", c=cs
                            )[:, :, :wb],
                            aps[:, : na * cs].rearrange(
                                "p (n c) -> p n c", c=cs
                            )[:, :, :wb],
                        )
                        attT[(w, g)] = att_sb
                    pe_backlog.append(tclos)
                # phase 3: out[k] = (sum_s ex[s] * x[s, :]) * rv
                for bwi in reversed(range(wb)):
                    k = b0 + bwi
                    holder = {}
                    for n0 in range(0, caps[k], 2):
                        def p3clos(n0=n0, k=k, bwi=bwi, w=w, holder=holder):
                            if n0 == 0:
                                # shares the atps ring (strict FIFO with
                                # the transposes) so two banks ping-pong
                                holder["ops"] = at_psum.tile(
                                    [1, H], F32, tag="atps", name="ops"
                                )
                            ops = holder["ops"]
                            for n in range(n0, min(n0 + 2, caps[k])):
                                col = (n % 4) * cs + bwi
                                nc.tensor.matmul(
                                    ops[:],
                                    attT[(w, n // 4)][:, col : col + 1],
                                    nat_sb[k][:, n * 512 : (n + 1) * 512],
                                    start=(n == 0),
                                    stop=(n == caps[k] - 1),
                                )
                        pe_backlog.append(p3clos)
                    def fin(k=k, bwi=bwi, rvT=rvT, holder=holder):
                        os_b = os_pool.tile([1, H], F32, tag="os")
                        # scale on ACT (frees the DVE drain path; the psum
                        # ring ping-pong hides the latency)
                        nc.scalar.activation(
                            os_b[:], holder["ops"][:],
                            mybir.ActivationFunctionType.Copy,
                            scale=rvT[0:1, bwi : bwi + 1],
                        )
                        nc.gpsimd.dma_start(out_d[k : k + 1, :], os_b[:])
                    pe_backlog.append(fin)

            for _rep in range(repeat):
                attT.clear()
                smxs.clear()
                for w, (b0, wb) in enumerate(WAVE_SPANS):
                    for bwi in range(wb):
                        trace_slot(w, b0, wb, bwi)
                    # softmax (DVE/ACT) fires immediately after the wave's
                    # last tanh; the PE part queues onto the backlog
                    finish_softmax(w, b0, wb)
                    push_finish_pe(w, b0, wb)
                while pe_backlog:
                    pe_backlog.popleft()()

    nc.finalize()
    return nc


_NC_CACHE = {}


def get_nc(caps, widths, repeat: int = 1) -> bass.Bass:
    key = (tuple(caps), tuple(widths), repeat)
    if key not in _NC_CACHE:
        _NC_CACHE[key] = build_nc(caps, widths, repeat=repeat)
    return _NC_CACHE[key]


def plan(lens):
    """Sort batches by length desc; slot k = ranks [8k, 8k+8), one per core.

    Window maxima of the descending sort minimize the summed per-slot caps
    (both the per-token p1/scores work and the 512*ceil(L/128) p3 term are
    monotone in L, so length-sorting is optimal for the combined cost).
    """
    lens = np.asarray(lens).reshape(B).astype(np.int64)
    order = np.argsort(-lens, kind="stable")
    assign = order.reshape(BPC, NCORES)          # [slot, core] -> batch
    widths = [int(lens[assign[k, 0]]) for k in range(BPC)]
    caps = [(w + 127) // 128 for w in widths]
    return caps, widths, assign


def make_in_maps(nn_outs, batch_lens, context, proj_w, proj_b, caps, widths,
                 assign):
    xf = np.asarray(nn_outs, dtype=np.float32)
    x_bf = xf.astype(ml_dtypes.bfloat16)
    x8 = (xf * SX).astype(E4M3)                  # [B, S, H] fp8
    lens = np.asarray(batch_lens).reshape(B).astype(np.int64)
    ctx_f = np.asarray(context, np.float32)
    Wf = np.asarray(proj_w, np.float32)          # [A, H]

    # --- W8 in SwInterleave ldweights layout, per (a-chunk, h-pair) ---
    W8 = (Wf * SW).astype(E4M3)
    # logical L[a,q][k,i,m] = W8[128a+m, 128(2q+i)+k]; storage st[k, 2j+i] =
    # L[k, i, 127-j] (pair-interleaved, columns reversed)
    Lf = np.ascontiguousarray(W8).reshape(AC, 128, 2, 2, 128)  # [a,m,q,i,k]
    T = Lf.transpose(0, 2, 4, 3, 1)                            # [a,q,k,i,m]
    st = T[..., ::-1].transpose(0, 1, 2, 4, 3)                 # [a,q,k,j,i]
    wt_host = np.ascontiguousarray(
        st.reshape(AC * 2, 128, 256).transpose(1, 0, 2).reshape(128, AC * 2 * 256)
    )

    # --- correction vector v = (W - W8)^T c, fp8, zero-padded stationary ---
    v = (Wf - W8.astype(np.float32) / SW).T @ ctx_f            # [H]
    v8 = (v * KV).astype(E4M3)
    vh = np.zeros((128, 2, WB, 2, VPAD), E4M3)
    vq = v8.reshape(2, 2, 128)                                 # [q, i, p]
    for bwi in range(WB):
        vh[:, :, bwi, :, bwi] = vq.transpose(2, 0, 1)
    v_host = np.ascontiguousarray(vh.reshape(128, 2 * WB * 2 * VPAD))

    # --- ctx (x SC) in the zero-padded wave layout ---
    ctx_c = (ctx_f * SC).reshape(AC, 128)
    ctx_host = np.zeros((128, AC, WB, VPAD), np.float32)
    for a in range(AC):
        for bw in range(WB):
            ctx_host[:, a, bw, bw] = ctx_c[a]
    ctx_host = np.ascontiguousarray(
        ctx_host.reshape(128, AC * WB * VPAD)
    ).astype(ml_dtypes.bfloat16)
    pb_host = np.ascontiguousarray(
        np.asarray(proj_b, np.float32).reshape(AC, 128).T
    )
    iota = np.arange(S)[None, :]

    xt_w = sum(4 * w for w in widths)
    nat_w = sum(c * 512 for c in caps)
    in_maps = []
    for c in range(NCORES):
        xt_all = np.empty((128, xt_w), E4M3)
        nat_all = np.empty((128, nat_w), ml_dtypes.bfloat16)
        mask = np.zeros((MW, len(WAVE_SPANS) * S), ml_dtypes.bfloat16)
        xo = no = 0
        for k in range(BPC):
            b = assign[k, c]
            W = widths[k]
            # xt8[p, q, i, s] = x8[b, s, 128(2q+i)+p]
            xt_all[:, xo : xo + 4 * W] = (
                x8[b, :W, :].T.reshape(2, 2, 128, W)
                .transpose(2, 0, 1, 3).reshape(128, 4 * W)
            )
            xo += 4 * W
            Wc = caps[k] * 128
            nat_all[:, no : no + caps[k] * 512] = (
                x_bf[b, :Wc, :].reshape(caps[k], 128, 512).transpose(1, 0, 2)
                .reshape(128, caps[k] * 512)
            )
            no += caps[k] * 512
        for w, (b0, wb) in enumerate(WAVE_SPANS):
            for bwi in range(wb):
                b = assign[b0 + bwi, c]
                mask[bwi, w * S : (w + 1) * S] = np.where(
                    iota[0] < lens[b], 0.0, -30000.0 * SC
                )
        in_maps.append(
            {
                "xt": xt_all,
                "nat": nat_all,
                "wt": wt_host,
                "ctx": ctx_host,
                "v8": v_host,
                "pb": pb_host,
                "mask": mask,
            }
        )
    return in_maps


def run(nn_outs, batch_lens, context, proj_w, proj_b, trace=False, repeat=1,
        **trace_kw):
    from concourse.bass_utils import run_bass_kernel_spmd

    caps, widths, assign = plan(batch_lens)
    nc = get_nc(caps, widths, repeat=repeat)
    in_maps = make_in_maps(
        nn_outs, batch_lens, context, proj_w, proj_b, caps, widths, assign
    )
    res = run_bass_kernel_spmd(
        nc, in_maps, list(range(NCORES)), trace=trace, **trace_kw
    )
    out = np.empty((B, H), np.float32)
    for c in range(NCORES):
        out[assign[:, c]] = res.results[c]["out"]
    return out, res


def kernel(nn_outs, batch_lens, context, proj_w, proj_b):
    out, _ = run(nn_outs, batch_lens, context, proj_w, proj_b, trace=False)
    return out
